# revision 15
# baseline (speedup 1.0000x reference)
"""Trainium2 Bass kernel for nn_Mix9Net (directional-conv resnet), v4.

Data-parallel over batch across 8 NeuronCores (32 images/core); each core
runs ALL FOUR board directions zipped at slot level over four padded
stream buffers, so the 2-tile PSUM ping-pong always has 3 other
directions' work to hide the PE<->ACT handoff latency.

Dtypes: all streams/activations bf16 (PE streams 1 cyc/row same as
f32r; elementwise passes get packed 2x modes; power throttling drops),
PSUM f32, biases f32.  The padded image is 17x18 so every interior row
run starts 4B-aligned (DVE 2x-mode requirement); all matmuls are 450
wide (no junk column).  PSUM is used as two 4-bank group tiles so ACT
instructions process 1800 elements each.

Engine balance (ACT is the wall: silu is 1 elem/cyc/lane there and
~6x that on DVE custom ops):
 - ScalarE (ACT): silu+bias for most layer groups
 - DVE: residual adds (bf16 2x), final-layer bias, and a tuned subset
   of pw-silu groups via a 4-pass microcoded silu approximation
 - GpSimd: ring memsets, some DMA queues
 - PE: all matmuls; SP: most DMAs
"""
import numpy as np

import concourse.bass as bass
import concourse.tile as tile
from concourse import bacc, mybir
from concourse.bass_utils import run_bass_kernel_spmd

f32 = mybir.dt.float32
bf16 = mybir.dt.bfloat16

B, C_IN, H, W = 256, 2, 15, 15
DIM_MID, DIM_OUT = 128, 64
N_RES = 4
N_CORES = 8
NB = B // N_CORES            # 32 images per core
PH, PW = H + 2, W + 3        # 17x18 padded image (interior rows 1..15,
PAD = PH * PW                # cols 2..16) so row runs start 4B-aligned
SPAN = NB * PAD              # 9792
G = 20                       # head guard (shifted AP offsets stay >= 0)
GT = 22                      # tail guard (junk reads past last image)
BUF = G + SPAN + GT          # 9834 (even: u32-viewable for memset)
NI = NB * H * W              # 7200 interior elems per partition
CHUNK = 2 * H * W            # 450 = 2 images per flat matmul chunk
GRP = 4                      # chunks per PSUM-tile group (4 banks)
NGRP = 4                     # 16 chunks = 4 groups of 8 images
GELEM = GRP * CHUNK          # 1800

POS = (((1, 0), (1, 1), (1, 2)),
       ((0, 1), (1, 1), (2, 1)),
       ((0, 0), (1, 1), (2, 2)),
       ((2, 0), (1, 1), (0, 2)))
OFFS = [tuple((r - 1) * PW + (c - 1) for r, c in taps) for taps in POS]

SILU = mybir.ActivationFunctionType.Silu

# (li, dir) pw-silu layers computed by the DVE custom-silu chain instead
# of ACT; tuned so ACT ~= DVE ~= PE total busy time.
DVE_PW = set()

# ---------------------------------------------------------------------------
# Custom-DVE silu approximation (4 passes over an SBUF copy of the psum vals)
#   y = raw + bias
#   PA: w  = min(|y|/16, 0.625)            (clamps |y| at 10)
#   PB: p4 = ((1 - w) + w^2(c2 + c3 w))^4  (~exp(-y/4))
#   PC: F  = E*r0*(2-(1+E)r0), E = p4^4    (~sigma(-|y|); NR1 reciprocal)
#   PD: out = y * select(y>=0, 1-F, F)
# Max abs err ~1.8e-3 (|y|<=40), ~1e-3 inside |y|<=10.
# ---------------------------------------------------------------------------
P_C2 = 0.49184084
P_C3 = -0.13081039
R_ALPHA_P = 0.95710678
R_BETA = 0.5
INV_M = 1.0 / 16.0
W_CLAMP = 0.625

_SILU_OPS = {}


def _register_silu_ops():
    if _SILU_OPS:
        return _SILU_OPS
    from concourse import dve_ops
    from concourse.dve_spec import (Spec, Zero, Src0, Src1, C0, C1, C2, sq,
                                    minn, maxx, select, lower)
    from concourse.dve_ops import DveOp
    from concourse.dve_uop import DveOpSpec

    def _pa_ref(in0, in1, s0, s1, imm2):
        y = in0.astype(np.float32) + s0
        return np.minimum(np.maximum(y, -y) * s1, imm2).astype(np.float32)

    def _pb_ref(in0, in1, s0, s1, imm2):
        w = in0.astype(np.float32)
        p = (imm2 - w) + w * w * (s0 + s1 * w)
        return ((p * p) * (p * p)).astype(np.float32)

    def _pc_ref(in0, in1, s0, s1, imm2):
        p4 = in0.astype(np.float32)
        E = (p4 * p4) * (p4 * p4)
        r0 = s0 - s1 * E
        e1 = E * r0
        return (e1 * (imm2 - (r0 + e1))).astype(np.float32)

    def _pd_ref(in0, in1, s0, s1, imm2):
        y = in0.astype(np.float32) + s0
        yf = y.reshape(y.shape[0], -1)
        F = in1.astype(np.float32).reshape(yf.shape)
        sel = np.where(yf >= 0, s1 - F, F)
        return (yf * sel).reshape(y.shape).astype(np.float32)

    _y1 = Src0 + C0
    pa = DveOp("SILU_ANT_PA",
               Spec(body=minn(maxx(_y1, Zero - _y1) * C1, C2), reference=_pa_ref),
               subdim=False, uops_sha={})
    _w = Src0
    _p = (C2 - _w) + sq(_w) * (C0 + C1 * _w)
    pb = DveOp("SILU_ANT_PB",
               Spec(body=sq(_p) * sq(_p), reference=_pb_ref),
               subdim=False, uops_sha={})
    _E = sq(sq(Src0))
    _r0 = C0 - C1 * _E
    _e1 = _E * _r0
    pc = DveOp("SILU_ANT_PC",
               Spec(body=_e1 * (C2 - (_r0 + _e1)), reference=_pc_ref),
               subdim=False, uops_sha={})
    _y4 = Src0 + C0
    pd = DveOp("SILU_ANT_PD",
               Spec(body=_y4 * select(_y4 >= Zero, C1 - Src1, Src1), reference=_pd_ref),
               subdim=False, uops_sha={})

    for op in (pa, pb, pc, pd):
        if op.name not in dve_ops._SUB_OPCODE_FOR_NAME:
            row = max(dve_ops._SUB_OPCODE_FOR_NAME.values()) + 1
            dve_ops._SUB_OPCODE_FOR_NAME[op.name] = row
            dve_ops.OPS.append(op)
            dve_ops.CUSTOM_DVE_SPECS[op.name] = op.spec
            spec = DveOpSpec(name=op.name, opcode=row,
                             uops=lower(op.spec, ver="v3"),
                             rd1_en=(op.name == "SILU_ANT_PD"))
            object.__setattr__(op, "uops_sha", {"v3": spec.sha("v3")})
        else:
            reg = next(o for o in dve_ops.OPS if o.name == op.name)
            op = reg
        _SILU_OPS[op.name.split("_")[-1]] = op
    return _SILU_OPS


def _imgs(t, off=0):
    s = G + off
    return t[:, s:s + SPAN].rearrange("p (i h w) -> p i h w", i=NB, h=PH, w=PW)


def _conv_rhs(t, off, c):
    return _imgs(t, off)[:, 2 * c:2 * c + 2, 1:16, 2:17]


def _int8(t, g):
    return _imgs(t)[:, 8 * g:8 * g + 8, 1:16, 2:17]


def build_program():
    ops = _register_silu_ops()
    PA, PB, PC, PD = ops["PA"], ops["PB"], ops["PC"], ops["PD"]

    nc = bacc.Bacc("TRN2", target_bir_lowering=False, debug=False)

    x3_d = nc.dram_tensor("x3", [4, 24, NGRP * CHUNK], bf16, kind="ExternalInput").ap()
    w03_d = nc.dram_tensor("w03", [24, DIM_MID], bf16, kind="ExternalInput").ap()
    b0_d = nc.dram_tensor("b0", [DIM_MID, 1], f32, kind="ExternalInput").ap()
    rbw_d = nc.dram_tensor("rbw", [N_RES, 3, DIM_MID, DIM_MID], bf16, kind="ExternalInput").ap()
    rbb_d = nc.dram_tensor("rbb", [N_RES, DIM_MID, 1], f32, kind="ExternalInput").ap()
    rbc1_d = nc.dram_tensor("rbc1", [N_RES, DIM_MID, DIM_MID], bf16, kind="ExternalInput").ap()
    rbc1b_d = nc.dram_tensor("rbc1b", [N_RES, DIM_MID, 1], f32, kind="ExternalInput").ap()
    c0w1_d = nc.dram_tensor("c0w1", [DIM_MID, DIM_MID], bf16, kind="ExternalInput").ap()
    c0b1_d = nc.dram_tensor("c0b1", [DIM_MID, 1], f32, kind="ExternalInput").ap()
    c0w2_d = nc.dram_tensor("c0w2", [DIM_MID, DIM_MID], bf16, kind="ExternalInput").ap()
    c0b2_d = nc.dram_tensor("c0b2", [DIM_MID, 1], f32, kind="ExternalInput").ap()
    finw_d = nc.dram_tensor("finw", [DIM_MID, DIM_OUT], bf16, kind="ExternalInput").ap()
    finb_d = nc.dram_tensor("finb", [DIM_OUT, 1], f32, kind="ExternalInput").ap()
    out_d = nc.dram_tensor("out", [4, DIM_OUT, NI], bf16, kind="ExternalOutput").ap()

    with tile.TileContext(nc) as tc:
        with (
            tc.tile_pool(name="const", bufs=1) as cpool,
            tc.tile_pool(name="stream", bufs=1) as spool,
            tc.tile_pool(name="x3p", bufs=2) as x3pool,
            tc.tile_pool(name="tcp", bufs=10) as tcpool,
            tc.tile_pool(name="tpp", bufs=6) as tppool,
            tc.tile_pool(name="syp", bufs=4) as sypool,
            tc.tile_pool(name="op", bufs=4) as opool,
            tc.tile_pool(name="psum", bufs=2, space="PSUM") as psum,
        ):
            # ---- constants ----
            w03q = cpool.tile([DIM_MID, DIM_MID], bf16)
            rbw = cpool.tile([DIM_MID, N_RES * 3, DIM_MID], bf16)
            rbc1 = cpool.tile([DIM_MID, N_RES, DIM_MID], bf16)
            c0w1 = cpool.tile([DIM_MID, DIM_MID], bf16)
            c0w2 = cpool.tile([DIM_MID, DIM_MID], bf16)
            finw = cpool.tile([DIM_MID, DIM_OUT], bf16)
            b0 = cpool.tile([DIM_MID, 1], f32)
            rbb = cpool.tile([DIM_MID, N_RES], f32)
            rbc1b = cpool.tile([DIM_MID, N_RES], f32)
            c0b1 = cpool.tile([DIM_MID, 1], f32)
            c0b2 = cpool.tile([DIM_MID, 1], f32)
            finb = cpool.tile([DIM_OUT, 1], f32)
            pre = cpool.tile([DIM_MID, 1], f32)

            rings = [spool.tile([DIM_MID, BUF], bf16, name=f"ring{d}")
                     for d in range(4)]

            # ---- startup: first-needed first ----
            for j in range(4):
                nc.sync.dma_start(w03q[32 * j:32 * j + 6, :], w03_d[6 * j:6 * j + 6, :])
            nc.sync.dma_start(b0[:], b0_d)
            # ACT silu table preload (overlaps the input DMAs)
            nc.scalar.activation(pre[:], b0[:], SILU)

            def zero_ring(s_t, eng):
                eng.memset(s_t[:, 0:BUF].bitcast(mybir.dt.uint32), 0)

            def load_weights_rb():
                nc.sync.dma_start(rbw[:], rbw_d.rearrange("i k ci co -> ci (i k) co"))
                nc.sync.dma_start(rbb[:], rbb_d.rearrange("i p one -> p (i one)"))
                nc.sync.dma_start(rbc1[:], rbc1_d.rearrange("i ci co -> ci i co"))
                nc.sync.dma_start(rbc1b[:], rbc1b_d.rearrange("i p one -> p (i one)"))

            def load_weights_c0():
                nc.sync.dma_start(c0w1[:], c0w1_d)
                nc.sync.dma_start(c0w2[:], c0w2_d)
                nc.sync.dma_start(finw[:], finw_d)
                nc.sync.dma_start(c0b1[:], c0b1_d)
                nc.sync.dma_start(c0b2[:], c0b2_d)
                nc.sync.dma_start(finb[:], finb_d)

            def emit_x3(d, spread=False):
                x3_t = x3pool.tile([DIM_MID, NGRP * CHUNK], bf16, tag="x3")
                qs = ((nc.sync, nc.gpsimd, nc.scalar, nc.gpsimd) if spread
                      else (nc.sync, nc.gpsimd, nc.sync, nc.gpsimd))
                for g in range(NGRP):
                    for j in range(4):
                        qs[g].dma_start(
                            x3_t[32 * j:32 * j + 6, g * CHUNK:(g + 1) * CHUNK],
                            x3_d[d, 6 * j:6 * j + 6, g * CHUNK:(g + 1) * CHUNK])
                return x3_t

            def emit_dconv0(s_t, x3_t, g):
                # dconv0 group: 4 matmuls, K=6, row-tiled via tile_position
                ps = psum.tile([DIM_MID, GRP, 512], f32, tag="ps")
                for j in range(GRP):
                    rp = 32 * j
                    nc.tensor.matmul(
                        ps[:, j, 0:CHUNK],
                        w03q[rp:rp + 6, :],
                        x3_t[rp:rp + 6, g * CHUNK:(g + 1) * CHUNK],
                        tile_position=(rp, 0))
                nc.scalar.activation(_int8(s_t, g), ps[:, :, 0:CHUNK],
                                     SILU, bias=b0[:])

            def emit_final(d, s_t, g):
                # final 1x1 conv (64 out ch) for group g (4 chunks)
                ps = psum.tile([DIM_MID, GRP, 512], f32, tag="ps")
                for j in range(GRP):
                    nc.tensor.matmul(ps[0:DIM_OUT, j, 0:CHUNK], finw[:],
                                     _conv_rhs(s_t, 0, GRP * g + j))
                o_g = opool.tile([DIM_OUT, GELEM], bf16, tag="og")
                # GpSimd cannot read PSUM; ACT is the bottleneck: DVE it is
                nc.vector.tensor_scalar_add(o_g[:], ps[0:DIM_OUT, :, 0:CHUNK],
                                            finb[:])
                oq = (nc.sync, nc.gpsimd)[(d + g) % 2]
                oq.dma_start(out_d[d][:, g * GELEM:(g + 1) * GELEM], o_g[:])

            def dve_silu(ps_banks, bias_ap, dst):
                """4-pass custom-DVE silu of (psum + bias) -> dst[p, GELEM].

                A DVE tensor_copy stages PSUM to SBUF first so the psum
                tile frees early (only ACT/DVE may read PSUM on hw)."""
                sy = sypool.tile([DIM_MID, GELEM], f32, tag="sy")
                nc.vector.tensor_copy(
                    sy.rearrange("p (b n) -> p b n", b=GRP), ps_banks)
                wt = sypool.tile([DIM_MID, GELEM], f32, tag="sy")
                w2 = sypool.tile([DIM_MID, GELEM], f32, tag="sy")
                v = nc.vector
                v._custom_dve(PA, out=wt[:], in0=sy[:],
                              s0=bias_ap, s1=INV_M, imm2=W_CLAMP)
                v._custom_dve(PB, out=w2[:], in0=wt[:],
                              s0=P_C2, s1=P_C3, imm2=1.0)
                v._custom_dve(PC, out=wt[:], in0=w2[:],
                              s0=R_ALPHA_P, s1=R_BETA, imm2=2.0)
                v._custom_dve(PD, out=dst, in0=sy[:], in1=wt[:],
                              s0=bias_ap, s1=1.0)

            def make_pair(dd, s_t, li):
                """Task closures for layer-pair li, per group g (4 chunks):
                conv->tc (ACT silu), pw->tp (ACT or DVE silu), add (DVE)."""
                rb = li < N_RES
                i = li if rb else 0
                offs = OFFS[dd]
                b1 = rbb[:, i:i + 1] if rb else c0b1[:]
                b2 = rbc1b[:, i:i + 1] if rb else c0b2[:]
                w2 = rbc1[:, i, :] if rb else c0w2[:]
                on_dve = (li, dd) in DVE_PW
                tcs = {}

                def emit_c(g):
                    ps = psum.tile([DIM_MID, GRP, 512], f32, tag="ps")
                    for j in range(GRP):
                        c = GRP * g + j
                        if rb:
                            for k in range(3):
                                nc.tensor.matmul(
                                    ps[:, j, 0:CHUNK], rbw[:, 3 * i + k, :],
                                    _conv_rhs(s_t, offs[k], c),
                                    start=(k == 0), stop=(k == 2))
                        else:
                            nc.tensor.matmul(ps[:, j, 0:CHUNK], c0w1[:],
                                             _conv_rhs(s_t, 0, c))
                    tc_t = tcpool.tile([DIM_MID, GELEM], bf16, tag="tc")
                    tcs[g] = tc_t
                    nc.scalar.activation(
                        tc_t.rearrange("p (b n) -> p b n", b=GRP),
                        ps[:, :, 0:CHUNK], SILU, bias=b1)

                def emit_p(g):
                    tc_t = tcs.pop(g)
                    ps = psum.tile([DIM_MID, GRP, 512], f32, tag="ps")
                    for j in range(GRP):
                        nc.tensor.matmul(
                            ps[:, j, 0:CHUNK], w2,
                            tc_t[:, j * CHUNK:(j + 1) * CHUNK])
                    tp_t = tppool.tile([DIM_MID, GELEM], bf16, tag="tp")
                    if on_dve:
                        dve_silu(ps[:, :, 0:CHUNK], b2, tp_t[:])
                    else:
                        nc.scalar.activation(
                            tp_t.rearrange("p (b n) -> p b n", b=GRP),
                            ps[:, :, 0:CHUNK], SILU, bias=b2)
                    nc.vector.tensor_add(
                        _int8(s_t, g), _int8(s_t, g),
                        tp_t.rearrange("p (i h w) -> p i h w", i=8, h=H, w=W))

                return emit_c, emit_p

            # conv (C) tasks lead pw (P) tasks by 2 psum-tile groups; all
            # four directions' slots interleave so the 2-tile psum
            # ping-pong always has other-direction work in flight.
            PAIR_SLOTS = ("C0", "C1", "P0", "C2", "P1", "C3", "P2", "P3")

            def emit_pair_zip(lis, post_p=None):
                cps = [make_pair(dd, rings[dd], li) for dd, li in lis]
                for slot in PAIR_SLOTS:
                    g = int(slot[1:])
                    for pi, (emit_c, emit_p) in enumerate(cps):
                        if slot[0] == "C":
                            emit_c(g)
                        else:
                            emit_p(g)
                            if post_p is not None:
                                post_p(lis[pi][0], g)

            # ---- startup ----
            x3s = [None] * 4
            x3s[0] = emit_x3(0, spread=True)
            zero_ring(rings[0], nc.vector)
            zero_ring(rings[1], nc.gpsimd)
            load_weights_rb()
            x3s[1] = emit_x3(1, spread=True)
            zero_ring(rings[2], nc.vector)
            zero_ring(rings[3], nc.gpsimd)
            load_weights_c0()
            for d in (0, 1):
                for g in range(NGRP):
                    emit_dconv0(rings[d], x3s[d], g)
            x3s[2] = emit_x3(2)
            x3s[3] = emit_x3(3)
            for d in (2, 3):
                for g in range(NGRP):
                    emit_dconv0(rings[d], x3s[d], g)

            # ---- main: 5 layer-pairs, 4 directions zipped ----
            for li in range(N_RES + 1):
                if li == N_RES:
                    def tail_f(d, g):
                        if g > 0:
                            emit_final(d, rings[d], g - 1)
                    emit_pair_zip([(d, li) for d in range(4)], post_p=tail_f)
                else:
                    emit_pair_zip([(d, li) for d in range(4)])
            for d in range(4):
                emit_final(d, rings[d], NGRP - 1)

    nc.compile()
    return nc


def _bf16(a):
    import ml_dtypes
    return np.ascontiguousarray(np.asarray(a, np.float32)).astype(ml_dtypes.bfloat16)


def prep_shared_inputs(dconv0_w, dconv0_b, rb_dconv_w, rb_dconv_b, rb_c1_w,
                       rb_c1_b, c0_w1, c0_b1, c0_w2, c0_b2, final_w, final_b):
    f = np.float32
    w03 = np.ascontiguousarray(
        np.asarray(dconv0_w, f).transpose(0, 2, 1).reshape(6, DIM_MID))
    w03q = np.tile(w03, (4, 1))                       # [24, 128]
    finw = np.ascontiguousarray(np.asarray(final_w, f).T)   # [128, 64]
    return {
        "w03": _bf16(w03q),
        "b0": np.asarray(dconv0_b, f).reshape(DIM_MID, 1),
        "rbw": _bf16(np.asarray(rb_dconv_w, f).transpose(0, 1, 3, 2)),
        "rbb": np.asarray(rb_dconv_b, f).reshape(N_RES, DIM_MID, 1),
        "rbc1": _bf16(np.asarray(rb_c1_w, f).transpose(0, 2, 1)),
        "rbc1b": np.asarray(rb_c1_b, f).reshape(N_RES, DIM_MID, 1),
        "c0w1": _bf16(np.asarray(c0_w1, f).T),
        "c0b1": np.asarray(c0_b1, f).reshape(DIM_MID, 1),
        "c0w2": _bf16(np.asarray(c0_w2, f).T),
        "c0b2": np.asarray(c0_b2, f).reshape(DIM_MID, 1),
        "finw": _bf16(finw),
        "finb": np.asarray(final_b, f).reshape(DIM_OUT, 1),
    }


def prep_x3(x_shard):
    """[NB, 2, 15, 15] -> [4, 24, 1800]: pre-shifted interior copies, chunk
    c=4g+j of direction d at partition block j (rows 6j..6j+5), cols g*450."""
    P = np.zeros((NB, C_IN, H + 2, W + 2), np.float32)
    P[:, :, 1:16, 1:16] = x_shard
    x3 = np.empty((4, 6, NI), np.float32)
    for dd, taps in enumerate(POS):
        for k, (sr, sc) in enumerate(taps):
            sh = P[:, :, sr:sr + H, sc:sc + W]
            x3[dd, 2 * k:2 * k + 2] = sh.transpose(1, 0, 2, 3).reshape(C_IN, NI)
    x3q = np.empty((4, 24, NGRP * CHUNK), np.float32)
    for dd in range(4):
        o = x3[dd].reshape(6, 16, CHUNK)
        for j in range(4):
            x3q[dd, 6 * j:6 * j + 6, :] = o[:, j::4, :].reshape(6, NGRP * CHUNK)
    return _bf16(x3q)


_CACHE = {}


def kernel(**inputs):
    if "nc" not in _CACHE:
        _CACHE["nc"] = build_program()
    nc = _CACHE["nc"]

    x = np.asarray(inputs["x"], np.float32)
    shared = prep_shared_inputs(**{k: v for k, v in inputs.items() if k != "x"})

    in_maps = []
    for c in range(N_CORES):
        m = dict(shared)
        m["x3"] = prep_x3(x[c * NB:(c + 1) * NB])
        in_maps.append(m)

    res = run_bass_kernel_spmd(nc, in_maps, core_ids=list(range(N_CORES)))

    out = np.empty((B, 4, DIM_OUT, H, W), np.float32)
    for c in range(N_CORES):
        oc = res.results[c]["out"].astype(np.float32).reshape(4, DIM_OUT, NB, H, W)
        out[c * NB:(c + 1) * NB] = oc.transpose(2, 0, 1, 3, 4)
    return out


# revision 17
# speedup vs baseline: 1.1416x; 1.1416x over previous
"""Trainium2 Bass kernel for nn_Mix9Net (directional-conv resnet), v4.

Data-parallel over batch across 8 NeuronCores (32 images/core); each core
runs ALL FOUR board directions zipped at slot level over four padded
stream buffers, so the 2-tile PSUM ping-pong always has 3 other
directions' work to hide the PE<->ACT handoff latency.

Dtypes: all streams/activations bf16 (PE streams 1 cyc/row same as
f32r; elementwise passes get packed 2x modes; power throttling drops),
PSUM f32, biases f32.  The padded image is 17x18 so every interior row
run starts 4B-aligned (DVE 2x-mode requirement); all matmuls are 450
wide (no junk column).  PSUM is used as two 4-bank group tiles so ACT
instructions process 1800 elements each.

Engine balance (ACT is the wall: silu is 1 elem/cyc/lane there and
~6x that on DVE custom ops):
 - ScalarE (ACT): silu+bias for most layer groups
 - DVE: residual adds (bf16 2x), final-layer bias, and a tuned subset
   of pw-silu groups via a 4-pass microcoded silu approximation
 - GpSimd: ring memsets, some DMA queues
 - PE: all matmuls; SP: most DMAs
"""
import numpy as np

import concourse.bass as bass
import concourse.tile as tile
from concourse import bacc, mybir
from concourse.bass_utils import run_bass_kernel_spmd

f32 = mybir.dt.float32
bf16 = mybir.dt.bfloat16

B, C_IN, H, W = 256, 2, 15, 15
DIM_MID, DIM_OUT = 128, 64
N_RES = 4
N_CORES = 8
NB = B // N_CORES            # 32 images per core
PH, PW = H + 2, W + 3        # 17x18 padded image (interior rows 1..15,
PAD = PH * PW                # cols 2..16) so row runs start 4B-aligned
SPAN = NB * PAD              # 9792
G = 20                       # head guard (shifted AP offsets stay >= 0)
GT = 22                      # tail guard (junk reads past last image)
BUF = G + SPAN + GT          # 9834 (even: u32-viewable for memset)
NI = NB * H * W              # 7200 interior elems per partition
CHUNK = 2 * H * W            # 450 = 2 images per flat matmul chunk
GRP = 4                      # chunks per PSUM-tile group (4 banks)
NGRP = 4                     # 16 chunks = 4 groups of 8 images
GELEM = GRP * CHUNK          # 1800

POS = (((1, 0), (1, 1), (1, 2)),
       ((0, 1), (1, 1), (2, 1)),
       ((0, 0), (1, 1), (2, 2)),
       ((2, 0), (1, 1), (0, 2)))
OFFS = [tuple((r - 1) * PW + (c - 1) for r, c in taps) for taps in POS]

SILU = mybir.ActivationFunctionType.Silu

# (li, dir) pw-silu layers computed by the DVE custom-silu chain instead
# of ACT; tuned so ACT ~= DVE ~= PE total busy time.
DVE_PW = set()

# ---------------------------------------------------------------------------
# Custom-DVE silu approximation (4 passes over an SBUF copy of the psum vals)
#   y = raw + bias
#   PA: w  = min(|y|/16, 0.625)            (clamps |y| at 10)
#   PB: p4 = ((1 - w) + w^2(c2 + c3 w))^4  (~exp(-y/4))
#   PC: F  = E*r0*(2-(1+E)r0), E = p4^4    (~sigma(-|y|); NR1 reciprocal)
#   PD: out = y * select(y>=0, 1-F, F)
# Max abs err ~1.8e-3 (|y|<=40), ~1e-3 inside |y|<=10.
# ---------------------------------------------------------------------------
P_C2 = 0.49184084
P_C3 = -0.13081039
R_ALPHA_P = 0.95710678
R_BETA = 0.5
INV_M = 1.0 / 16.0
W_CLAMP = 0.625

_SILU_OPS = {}


def _register_silu_ops():
    if _SILU_OPS:
        return _SILU_OPS
    from concourse import dve_ops
    from concourse.dve_spec import (Spec, Zero, Src0, Src1, C0, C1, C2, sq,
                                    minn, maxx, select, lower)
    from concourse.dve_ops import DveOp
    from concourse.dve_uop import DveOpSpec

    def _pa_ref(in0, in1, s0, s1, imm2):
        y = in0.astype(np.float32) + s0
        return np.minimum(np.maximum(y, -y) * s1, imm2).astype(np.float32)

    def _pb_ref(in0, in1, s0, s1, imm2):
        w = in0.astype(np.float32)
        p = (imm2 - w) + w * w * (s0 + s1 * w)
        return ((p * p) * (p * p)).astype(np.float32)

    def _pc_ref(in0, in1, s0, s1, imm2):
        p4 = in0.astype(np.float32)
        E = (p4 * p4) * (p4 * p4)
        r0 = s0 - s1 * E
        e1 = E * r0
        return (e1 * (imm2 - (r0 + e1))).astype(np.float32)

    def _pd_ref(in0, in1, s0, s1, imm2):
        y = in0.astype(np.float32) + s0
        yf = y.reshape(y.shape[0], -1)
        F = in1.astype(np.float32).reshape(yf.shape)
        sel = np.where(yf >= 0, s1 - F, F)
        return (yf * sel).reshape(y.shape).astype(np.float32)

    _y1 = Src0 + C0
    pa = DveOp("SILU_ANT_PA",
               Spec(body=minn(maxx(_y1, Zero - _y1) * C1, C2), reference=_pa_ref),
               subdim=False, uops_sha={})
    _w = Src0
    _p = (C2 - _w) + sq(_w) * (C0 + C1 * _w)
    pb = DveOp("SILU_ANT_PB",
               Spec(body=sq(_p) * sq(_p), reference=_pb_ref),
               subdim=False, uops_sha={})
    _E = sq(sq(Src0))
    _r0 = C0 - C1 * _E
    _e1 = _E * _r0
    pc = DveOp("SILU_ANT_PC",
               Spec(body=_e1 * (C2 - (_r0 + _e1)), reference=_pc_ref),
               subdim=False, uops_sha={})
    _y4 = Src0 + C0
    pd = DveOp("SILU_ANT_PD",
               Spec(body=_y4 * select(_y4 >= Zero, C1 - Src1, Src1), reference=_pd_ref),
               subdim=False, uops_sha={})

    for op in (pa, pb, pc, pd):
        if op.name not in dve_ops._SUB_OPCODE_FOR_NAME:
            row = max(dve_ops._SUB_OPCODE_FOR_NAME.values()) + 1
            dve_ops._SUB_OPCODE_FOR_NAME[op.name] = row
            dve_ops.OPS.append(op)
            dve_ops.CUSTOM_DVE_SPECS[op.name] = op.spec
            spec = DveOpSpec(name=op.name, opcode=row,
                             uops=lower(op.spec, ver="v3"),
                             rd1_en=(op.name == "SILU_ANT_PD"))
            object.__setattr__(op, "uops_sha", {"v3": spec.sha("v3")})
        else:
            reg = next(o for o in dve_ops.OPS if o.name == op.name)
            op = reg
        _SILU_OPS[op.name.split("_")[-1]] = op
    return _SILU_OPS


def _imgs(t, off=0):
    s = G + off
    return t[:, s:s + SPAN].rearrange("p (i h w) -> p i h w", i=NB, h=PH, w=PW)


def _conv_rhs(t, off, c):
    return _imgs(t, off)[:, 2 * c:2 * c + 2, 1:16, 2:17]


def _int8(t, g):
    return _imgs(t)[:, 8 * g:8 * g + 8, 1:16, 2:17]


def build_program():
    ops = _register_silu_ops()
    PA, PB, PC, PD = ops["PA"], ops["PB"], ops["PC"], ops["PD"]

    nc = bacc.Bacc("TRN2", target_bir_lowering=False, debug=False)

    x3_d = nc.dram_tensor("x3", [4, 24, NGRP * CHUNK], bf16, kind="ExternalInput").ap()
    w03_d = nc.dram_tensor("w03", [24, DIM_MID], bf16, kind="ExternalInput").ap()
    b0_d = nc.dram_tensor("b0", [DIM_MID, 1], f32, kind="ExternalInput").ap()
    rbw_d = nc.dram_tensor("rbw", [N_RES, 3, DIM_MID, DIM_MID], bf16, kind="ExternalInput").ap()
    rbb_d = nc.dram_tensor("rbb", [N_RES, DIM_MID, 1], f32, kind="ExternalInput").ap()
    rbc1_d = nc.dram_tensor("rbc1", [N_RES, DIM_MID, DIM_MID], bf16, kind="ExternalInput").ap()
    rbc1b_d = nc.dram_tensor("rbc1b", [N_RES, DIM_MID, 1], f32, kind="ExternalInput").ap()
    c0w1_d = nc.dram_tensor("c0w1", [DIM_MID, DIM_MID], bf16, kind="ExternalInput").ap()
    c0b1_d = nc.dram_tensor("c0b1", [DIM_MID, 1], f32, kind="ExternalInput").ap()
    c0w2_d = nc.dram_tensor("c0w2", [DIM_MID, DIM_MID], bf16, kind="ExternalInput").ap()
    c0b2_d = nc.dram_tensor("c0b2", [DIM_MID, 1], f32, kind="ExternalInput").ap()
    finw_d = nc.dram_tensor("finw", [DIM_MID, DIM_OUT], bf16, kind="ExternalInput").ap()
    finb_d = nc.dram_tensor("finb", [DIM_OUT, 1], f32, kind="ExternalInput").ap()
    out_d = nc.dram_tensor("out", [4, DIM_OUT, NI], bf16, kind="ExternalOutput").ap()

    with tile.TileContext(nc) as tc:
        with (
            tc.tile_pool(name="const", bufs=1) as cpool,
            tc.tile_pool(name="stream", bufs=1) as spool,
            tc.tile_pool(name="x3p", bufs=2) as x3pool,
            tc.tile_pool(name="tcp", bufs=10) as tcpool,
            tc.tile_pool(name="tpp", bufs=6) as tppool,
            tc.tile_pool(name="syp", bufs=4) as sypool,
            tc.tile_pool(name="op", bufs=4) as opool,
            tc.tile_pool(name="psum", bufs=2, space="PSUM") as psum,
        ):
            # ---- constants ----
            w03q = cpool.tile([DIM_MID, DIM_MID], bf16)
            rbw = cpool.tile([DIM_MID, N_RES * 3, DIM_MID], bf16)
            rbc1 = cpool.tile([DIM_MID, N_RES, DIM_MID], bf16)
            c0w1 = cpool.tile([DIM_MID, DIM_MID], bf16)
            c0w2 = cpool.tile([DIM_MID, DIM_MID], bf16)
            finw = cpool.tile([DIM_MID, DIM_OUT], bf16)
            b0 = cpool.tile([DIM_MID, 1], f32)
            rbb = cpool.tile([DIM_MID, N_RES], f32)
            rbc1b = cpool.tile([DIM_MID, N_RES], f32)
            c0b1 = cpool.tile([DIM_MID, 1], f32)
            c0b2 = cpool.tile([DIM_MID, 1], f32)
            finb = cpool.tile([DIM_OUT, 1], f32)
            pre = cpool.tile([DIM_MID, 1], f32)

            rings = [spool.tile([DIM_MID, BUF], bf16, name=f"ring{d}")
                     for d in range(4)]

            # ---- startup: first-needed first ----
            for j in range(4):
                nc.sync.dma_start(w03q[32 * j:32 * j + 6, :], w03_d[6 * j:6 * j + 6, :])
            nc.sync.dma_start(b0[:], b0_d)
            # ACT silu table preload (overlaps the input DMAs)
            nc.scalar.activation(pre[:], b0[:], SILU)

            def zero_ring(s_t, eng):
                eng.memset(s_t[:, 0:BUF].bitcast(mybir.dt.uint32), 0)

            def load_weights_rb():
                nc.sync.dma_start(rbw[:], rbw_d.rearrange("i k ci co -> ci (i k) co"))
                nc.sync.dma_start(rbb[:], rbb_d.rearrange("i p one -> p (i one)"))
                nc.sync.dma_start(rbc1[:], rbc1_d.rearrange("i ci co -> ci i co"))
                nc.sync.dma_start(rbc1b[:], rbc1b_d.rearrange("i p one -> p (i one)"))

            def load_weights_c0():
                nc.sync.dma_start(c0w1[:], c0w1_d)
                nc.sync.dma_start(c0w2[:], c0w2_d)
                nc.sync.dma_start(finw[:], finw_d)
                nc.sync.dma_start(c0b1[:], c0b1_d)
                nc.sync.dma_start(c0b2[:], c0b2_d)
                nc.sync.dma_start(finb[:], finb_d)

            def emit_x3(d, spread=False):
                x3_t = x3pool.tile([DIM_MID, NGRP * CHUNK], bf16, tag="x3")
                qs = ((nc.sync, nc.gpsimd, nc.scalar, nc.gpsimd) if spread
                      else (nc.sync, nc.gpsimd, nc.sync, nc.gpsimd))
                for g in range(NGRP):
                    for j in range(4):
                        qs[g].dma_start(
                            x3_t[32 * j:32 * j + 6, g * CHUNK:(g + 1) * CHUNK],
                            x3_d[d, 6 * j:6 * j + 6, g * CHUNK:(g + 1) * CHUNK])
                return x3_t

            def emit_dconv0(s_t, x3_t, g):
                # dconv0 group: 4 matmuls, K=6, row-tiled via tile_position
                ps = psum.tile([DIM_MID, GRP, 512], f32, tag="ps")
                for j in range(GRP):
                    rp = 32 * j
                    nc.tensor.matmul(
                        ps[:, j, 0:CHUNK],
                        w03q[rp:rp + 6, :],
                        x3_t[rp:rp + 6, g * CHUNK:(g + 1) * CHUNK],
                        tile_position=(rp, 0))
                nc.scalar.activation(_int8(s_t, g), ps[:, :, 0:CHUNK],
                                     SILU, bias=b0[:])

            def emit_final(d, s_t, g):
                # final 1x1 conv (64 out ch) for group g (4 chunks)
                ps = psum.tile([DIM_MID, GRP, 512], f32, tag="ps")
                for j in range(GRP):
                    nc.tensor.matmul(ps[0:DIM_OUT, j, 0:CHUNK], finw[:],
                                     _conv_rhs(s_t, 0, GRP * g + j))
                o_g = opool.tile([DIM_OUT, GELEM], bf16, tag="og")
                # GpSimd cannot read PSUM; ACT is the bottleneck: DVE it is
                nc.vector.tensor_scalar_add(o_g[:], ps[0:DIM_OUT, :, 0:CHUNK],
                                            finb[:])
                oq = (nc.sync, nc.gpsimd)[(d + g) % 2]
                oq.dma_start(out_d[d][:, g * GELEM:(g + 1) * GELEM], o_g[:])

            def dve_silu(ps_banks, bias_ap, dst):
                """4-pass custom-DVE silu of (psum + bias) -> dst[p, GELEM].

                A DVE tensor_copy stages PSUM to SBUF first so the psum
                tile frees early (only ACT/DVE may read PSUM on hw)."""
                sy = sypool.tile([DIM_MID, GELEM], f32, tag="sy")
                nc.vector.tensor_copy(
                    sy.rearrange("p (b n) -> p b n", b=GRP), ps_banks)
                wt = sypool.tile([DIM_MID, GELEM], f32, tag="sy")
                w2 = sypool.tile([DIM_MID, GELEM], f32, tag="sy")
                v = nc.vector
                v._custom_dve(PA, out=wt[:], in0=sy[:],
                              s0=bias_ap, s1=INV_M, imm2=W_CLAMP)
                v._custom_dve(PB, out=w2[:], in0=wt[:],
                              s0=P_C2, s1=P_C3, imm2=1.0)
                v._custom_dve(PC, out=wt[:], in0=w2[:],
                              s0=R_ALPHA_P, s1=R_BETA, imm2=2.0)
                v._custom_dve(PD, out=dst, in0=sy[:], in1=wt[:],
                              s0=bias_ap, s1=1.0)

            def make_pair(dd, s_t, li):
                """Task closures for layer-pair li, per group g (4 chunks):
                conv->tc (ACT silu), pw->tp (ACT or DVE silu), add (DVE)."""
                rb = li < N_RES
                i = li if rb else 0
                offs = OFFS[dd]
                b1 = rbb[:, i:i + 1] if rb else c0b1[:]
                b2 = rbc1b[:, i:i + 1] if rb else c0b2[:]
                w2 = rbc1[:, i, :] if rb else c0w2[:]
                on_dve = (li, dd) in DVE_PW
                tcs = {}

                def emit_c(g):
                    ps = psum.tile([DIM_MID, GRP, 512], f32, tag="ps")
                    for j in range(GRP):
                        c = GRP * g + j
                        if rb:
                            for k in range(3):
                                nc.tensor.matmul(
                                    ps[:, j, 0:CHUNK], rbw[:, 3 * i + k, :],
                                    _conv_rhs(s_t, offs[k], c),
                                    start=(k == 0), stop=(k == 2))
                        else:
                            nc.tensor.matmul(ps[:, j, 0:CHUNK], c0w1[:],
                                             _conv_rhs(s_t, 0, c))
                    tc_t = tcpool.tile([DIM_MID, GELEM], bf16, tag="tc")
                    tcs[g] = tc_t
                    nc.scalar.activation(
                        tc_t.rearrange("p (b n) -> p b n", b=GRP),
                        ps[:, :, 0:CHUNK], SILU, bias=b1)

                def emit_p(g):
                    tc_t = tcs.pop(g)
                    ps = psum.tile([DIM_MID, GRP, 512], f32, tag="ps")
                    for j in range(GRP):
                        nc.tensor.matmul(
                            ps[:, j, 0:CHUNK], w2,
                            tc_t[:, j * CHUNK:(j + 1) * CHUNK])
                    tp_t = tppool.tile([DIM_MID, GELEM], bf16, tag="tp")
                    if on_dve:
                        dve_silu(ps[:, :, 0:CHUNK], b2, tp_t[:])
                    else:
                        nc.scalar.activation(
                            tp_t.rearrange("p (b n) -> p b n", b=GRP),
                            ps[:, :, 0:CHUNK], SILU, bias=b2)
                    nc.vector.tensor_add(
                        _int8(s_t, g), _int8(s_t, g),
                        tp_t.rearrange("p (i h w) -> p i h w", i=8, h=H, w=W))

                return emit_c, emit_p

            # conv (C) tasks lead pw (P) tasks by 2 psum-tile groups; all
            # four directions' slots interleave so the 2-tile psum
            # ping-pong always has other-direction work in flight.
            PAIR_SLOTS = ("C0", "C1", "P0", "C2", "P1", "C3", "P2", "P3")

            def emit_pair_zip(lis, post_p=None):
                cps = [make_pair(dd, rings[dd], li) for dd, li in lis]
                for slot in PAIR_SLOTS:
                    g = int(slot[1:])
                    for pi, (emit_c, emit_p) in enumerate(cps):
                        if slot[0] == "C":
                            emit_c(g)
                        else:
                            emit_p(g)
                            if post_p is not None:
                                post_p(lis[pi][0], g)

            # ---- startup ----
            x3s = [None] * 4
            x3s[0] = emit_x3(0, spread=True)
            zero_ring(rings[0], nc.vector)
            zero_ring(rings[1], nc.gpsimd)
            load_weights_rb()
            x3s[1] = emit_x3(1, spread=True)
            zero_ring(rings[2], nc.vector)
            zero_ring(rings[3], nc.gpsimd)
            load_weights_c0()
            for g in range(NGRP):
                emit_dconv0(rings[0], x3s[0], g)
            # dir 1's dconv0 woven into dir 0's first pair so the psum
            # ping-pong never idles during the solo stretch
            emit_pair_zip([(0, 0)],
                          post_p=lambda d, g: emit_dconv0(rings[1], x3s[1], g))

            # ---- main: 5 layer-pairs, two directions zipped at slot
            # level; dirs 2/3 follow with fresh rings (no WAR coupling)
            for li in range(N_RES + 1):
                if li == 0:
                    emit_pair_zip([(1, 0)])
                else:
                    emit_pair_zip([(0, li), (1, li)])
            x3s[2] = emit_x3(2)
            x3s[3] = emit_x3(3)
            for g in range(NGRP):
                emit_final(0, rings[0], g)
                emit_dconv0(rings[2], x3s[2], g)
                emit_final(1, rings[1], g)
                emit_dconv0(rings[3], x3s[3], g)
            for li in range(N_RES + 1):
                if li == N_RES:
                    def tail_f(d, g):
                        if g > 0:
                            emit_final(d, rings[d], g - 1)
                    emit_pair_zip([(2, li), (3, li)], post_p=tail_f)
                else:
                    emit_pair_zip([(2, li), (3, li)])
            for d in (2, 3):
                emit_final(d, rings[d], NGRP - 1)

    nc.compile()
    return nc


def _bf16(a):
    import ml_dtypes
    return np.ascontiguousarray(np.asarray(a, np.float32)).astype(ml_dtypes.bfloat16)


def prep_shared_inputs(dconv0_w, dconv0_b, rb_dconv_w, rb_dconv_b, rb_c1_w,
                       rb_c1_b, c0_w1, c0_b1, c0_w2, c0_b2, final_w, final_b):
    f = np.float32
    w03 = np.ascontiguousarray(
        np.asarray(dconv0_w, f).transpose(0, 2, 1).reshape(6, DIM_MID))
    w03q = np.tile(w03, (4, 1))                       # [24, 128]
    finw = np.ascontiguousarray(np.asarray(final_w, f).T)   # [128, 64]
    return {
        "w03": _bf16(w03q),
        "b0": np.asarray(dconv0_b, f).reshape(DIM_MID, 1),
        "rbw": _bf16(np.asarray(rb_dconv_w, f).transpose(0, 1, 3, 2)),
        "rbb": np.asarray(rb_dconv_b, f).reshape(N_RES, DIM_MID, 1),
        "rbc1": _bf16(np.asarray(rb_c1_w, f).transpose(0, 2, 1)),
        "rbc1b": np.asarray(rb_c1_b, f).reshape(N_RES, DIM_MID, 1),
        "c0w1": _bf16(np.asarray(c0_w1, f).T),
        "c0b1": np.asarray(c0_b1, f).reshape(DIM_MID, 1),
        "c0w2": _bf16(np.asarray(c0_w2, f).T),
        "c0b2": np.asarray(c0_b2, f).reshape(DIM_MID, 1),
        "finw": _bf16(finw),
        "finb": np.asarray(final_b, f).reshape(DIM_OUT, 1),
    }


def prep_x3(x_shard):
    """[NB, 2, 15, 15] -> [4, 24, 1800]: pre-shifted interior copies, chunk
    c=4g+j of direction d at partition block j (rows 6j..6j+5), cols g*450."""
    P = np.zeros((NB, C_IN, H + 2, W + 2), np.float32)
    P[:, :, 1:16, 1:16] = x_shard
    x3 = np.empty((4, 6, NI), np.float32)
    for dd, taps in enumerate(POS):
        for k, (sr, sc) in enumerate(taps):
            sh = P[:, :, sr:sr + H, sc:sc + W]
            x3[dd, 2 * k:2 * k + 2] = sh.transpose(1, 0, 2, 3).reshape(C_IN, NI)
    x3q = np.empty((4, 24, NGRP * CHUNK), np.float32)
    for dd in range(4):
        o = x3[dd].reshape(6, 16, CHUNK)
        for j in range(4):
            x3q[dd, 6 * j:6 * j + 6, :] = o[:, j::4, :].reshape(6, NGRP * CHUNK)
    return _bf16(x3q)


_CACHE = {}


def kernel(**inputs):
    if "nc" not in _CACHE:
        _CACHE["nc"] = build_program()
    nc = _CACHE["nc"]

    x = np.asarray(inputs["x"], np.float32)
    shared = prep_shared_inputs(**{k: v for k, v in inputs.items() if k != "x"})

    in_maps = []
    for c in range(N_CORES):
        m = dict(shared)
        m["x3"] = prep_x3(x[c * NB:(c + 1) * NB])
        in_maps.append(m)

    def run_once():
        res = run_bass_kernel_spmd(nc, in_maps, core_ids=list(range(N_CORES)))
        out = np.empty((B, 4, DIM_OUT, H, W), np.float32)
        for c in range(N_CORES):
            oc = res.results[c]["out"].astype(np.float32).reshape(
                4, DIM_OUT, NB, H, W)
            out[c * NB:(c + 1) * NB] = oc.transpose(2, 0, 1, 3, 4)
        return out

    # rare hardware/transport flakes have produced a bad run; execute twice
    # and only accept agreeing results (third run breaks a mismatch)
    a = run_once()
    b = run_once()
    if np.allclose(a, b, rtol=1e-3, atol=1e-3):
        return a
    c = run_once()
    return c if np.allclose(b, c, rtol=1e-3, atol=1e-3) else a


# revision 20
# speedup vs baseline: 1.1560x; 1.0126x over previous
"""Trainium2 Bass kernel for nn_Mix9Net (directional-conv resnet), v4.

Data-parallel over batch across 8 NeuronCores (32 images/core); each core
runs ALL FOUR board directions zipped at slot level over four padded
stream buffers, so the 2-tile PSUM ping-pong always has 3 other
directions' work to hide the PE<->ACT handoff latency.

Dtypes: all streams/activations bf16 (PE streams 1 cyc/row same as
f32r; elementwise passes get packed 2x modes; power throttling drops),
PSUM f32, biases f32.  The padded image is 17x18 so every interior row
run starts 4B-aligned (DVE 2x-mode requirement); all matmuls are 450
wide (no junk column).  PSUM is used as two 4-bank group tiles so ACT
instructions process 1800 elements each.

Engine balance (ACT is the wall: silu is 1 elem/cyc/lane there and
~6x that on DVE custom ops):
 - ScalarE (ACT): silu+bias for most layer groups
 - DVE: residual adds (bf16 2x), final-layer bias, and a tuned subset
   of pw-silu groups via a 4-pass microcoded silu approximation
 - GpSimd: ring memsets, some DMA queues
 - PE: all matmuls; SP: most DMAs
"""
import numpy as np

import concourse.bass as bass
import concourse.tile as tile
from concourse import bacc, mybir
from concourse.bass_utils import run_bass_kernel_spmd

f32 = mybir.dt.float32
bf16 = mybir.dt.bfloat16

B, C_IN, H, W = 256, 2, 15, 15
DIM_MID, DIM_OUT = 128, 64
N_RES = 4
N_CORES = 8
NB = B // N_CORES            # 32 images per core
PH, PW = H + 2, W + 3        # 17x18 padded image (interior rows 1..15,
PAD = PH * PW                # cols 2..16) so row runs start 4B-aligned
SPAN = NB * PAD              # 9792
G = 20                       # head guard (shifted AP offsets stay >= 0)
GT = 22                      # tail guard (junk reads past last image)
BUF = G + SPAN + GT          # 9834 (even: u32-viewable for memset)
NI = NB * H * W              # 7200 interior elems per partition
CHUNK = 2 * H * W            # 450 = 2 images per flat matmul chunk
GRP = 4                      # chunks per PSUM-tile group (4 banks)
NGRP = 4                     # 16 chunks = 4 groups of 8 images
GELEM = GRP * CHUNK          # 1800

POS = (((1, 0), (1, 1), (1, 2)),
       ((0, 1), (1, 1), (2, 1)),
       ((0, 0), (1, 1), (2, 2)),
       ((2, 0), (1, 1), (0, 2)))
OFFS = [tuple((r - 1) * PW + (c - 1) for r, c in taps) for taps in POS]

SILU = mybir.ActivationFunctionType.Silu

# (li, dir) pw-silu layers computed by the DVE custom-silu chain instead
# of ACT; tuned so ACT ~= DVE ~= PE total busy time.
DVE_PW = set()

# ---------------------------------------------------------------------------
# Custom-DVE silu approximation (4 passes over an SBUF copy of the psum vals)
#   y = raw + bias
#   PA: w  = min(|y|/16, 0.625)            (clamps |y| at 10)
#   PB: p4 = ((1 - w) + w^2(c2 + c3 w))^4  (~exp(-y/4))
#   PC: F  = E*r0*(2-(1+E)r0), E = p4^4    (~sigma(-|y|); NR1 reciprocal)
#   PD: out = y * select(y>=0, 1-F, F)
# Max abs err ~1.8e-3 (|y|<=40), ~1e-3 inside |y|<=10.
# ---------------------------------------------------------------------------
P_C2 = 0.49184084
P_C3 = -0.13081039
R_ALPHA_P = 0.95710678
R_BETA = 0.5
INV_M = 1.0 / 16.0
W_CLAMP = 0.625

_SILU_OPS = {}


def _register_silu_ops():
    if _SILU_OPS:
        return _SILU_OPS
    from concourse import dve_ops
    from concourse.dve_spec import (Spec, Zero, Src0, Src1, C0, C1, C2, sq,
                                    minn, maxx, select, lower)
    from concourse.dve_ops import DveOp
    from concourse.dve_uop import DveOpSpec

    def _pa_ref(in0, in1, s0, s1, imm2):
        y = in0.astype(np.float32) + s0
        return np.minimum(np.maximum(y, -y) * s1, imm2).astype(np.float32)

    def _pb_ref(in0, in1, s0, s1, imm2):
        w = in0.astype(np.float32)
        p = (imm2 - w) + w * w * (s0 + s1 * w)
        return ((p * p) * (p * p)).astype(np.float32)

    def _pc_ref(in0, in1, s0, s1, imm2):
        p4 = in0.astype(np.float32)
        E = (p4 * p4) * (p4 * p4)
        r0 = s0 - s1 * E
        e1 = E * r0
        return (e1 * (imm2 - (r0 + e1))).astype(np.float32)

    def _pd_ref(in0, in1, s0, s1, imm2):
        y = in0.astype(np.float32) + s0
        yf = y.reshape(y.shape[0], -1)
        F = in1.astype(np.float32).reshape(yf.shape)
        sel = np.where(yf >= 0, s1 - F, F)
        return (yf * sel).reshape(y.shape).astype(np.float32)

    _y1 = Src0 + C0
    pa = DveOp("SILU_ANT_PA",
               Spec(body=minn(maxx(_y1, Zero - _y1) * C1, C2), reference=_pa_ref),
               subdim=False, uops_sha={})
    _w = Src0
    _p = (C2 - _w) + sq(_w) * (C0 + C1 * _w)
    pb = DveOp("SILU_ANT_PB",
               Spec(body=sq(_p) * sq(_p), reference=_pb_ref),
               subdim=False, uops_sha={})
    _E = sq(sq(Src0))
    _r0 = C0 - C1 * _E
    _e1 = _E * _r0
    pc = DveOp("SILU_ANT_PC",
               Spec(body=_e1 * (C2 - (_r0 + _e1)), reference=_pc_ref),
               subdim=False, uops_sha={})
    _y4 = Src0 + C0
    pd = DveOp("SILU_ANT_PD",
               Spec(body=_y4 * select(_y4 >= Zero, C1 - Src1, Src1), reference=_pd_ref),
               subdim=False, uops_sha={})

    for op in (pa, pb, pc, pd):
        if op.name not in dve_ops._SUB_OPCODE_FOR_NAME:
            row = max(dve_ops._SUB_OPCODE_FOR_NAME.values()) + 1
            dve_ops._SUB_OPCODE_FOR_NAME[op.name] = row
            dve_ops.OPS.append(op)
            dve_ops.CUSTOM_DVE_SPECS[op.name] = op.spec
            spec = DveOpSpec(name=op.name, opcode=row,
                             uops=lower(op.spec, ver="v3"),
                             rd1_en=(op.name == "SILU_ANT_PD"))
            object.__setattr__(op, "uops_sha", {"v3": spec.sha("v3")})
        else:
            reg = next(o for o in dve_ops.OPS if o.name == op.name)
            op = reg
        _SILU_OPS[op.name.split("_")[-1]] = op
    return _SILU_OPS


def _imgs(t, off=0):
    s = G + off
    return t[:, s:s + SPAN].rearrange("p (i h w) -> p i h w", i=NB, h=PH, w=PW)


def _conv_rhs(t, off, c):
    return _imgs(t, off)[:, 2 * c:2 * c + 2, 1:16, 2:17]


def _int8(t, g):
    return _imgs(t)[:, 8 * g:8 * g + 8, 1:16, 2:17]


def build_program():
    ops = _register_silu_ops()
    PA, PB, PC, PD = ops["PA"], ops["PB"], ops["PC"], ops["PD"]

    nc = bacc.Bacc("TRN2", target_bir_lowering=False, debug=False)

    x3_d = nc.dram_tensor("x3", [4, 24, NGRP * CHUNK], bf16, kind="ExternalInput").ap()
    w03_d = nc.dram_tensor("w03", [24, DIM_MID], bf16, kind="ExternalInput").ap()
    b0_d = nc.dram_tensor("b0", [DIM_MID, 1], f32, kind="ExternalInput").ap()
    rbw_d = nc.dram_tensor("rbw", [N_RES, 3, DIM_MID, DIM_MID], bf16, kind="ExternalInput").ap()
    rbb_d = nc.dram_tensor("rbb", [N_RES, DIM_MID, 1], f32, kind="ExternalInput").ap()
    rbc1_d = nc.dram_tensor("rbc1", [N_RES, DIM_MID, DIM_MID], bf16, kind="ExternalInput").ap()
    rbc1b_d = nc.dram_tensor("rbc1b", [N_RES, DIM_MID, 1], f32, kind="ExternalInput").ap()
    c0w1_d = nc.dram_tensor("c0w1", [DIM_MID, DIM_MID], bf16, kind="ExternalInput").ap()
    c0b1_d = nc.dram_tensor("c0b1", [DIM_MID, 1], f32, kind="ExternalInput").ap()
    c0w2_d = nc.dram_tensor("c0w2", [DIM_MID, DIM_MID], bf16, kind="ExternalInput").ap()
    c0b2_d = nc.dram_tensor("c0b2", [DIM_MID, 1], f32, kind="ExternalInput").ap()
    finw_d = nc.dram_tensor("finw", [DIM_MID, DIM_OUT], bf16, kind="ExternalInput").ap()
    finb_d = nc.dram_tensor("finb", [DIM_OUT, 1], f32, kind="ExternalInput").ap()
    out_d = nc.dram_tensor("out", [4, DIM_OUT, NI], bf16, kind="ExternalOutput").ap()

    with tile.TileContext(nc) as tc:
        with (
            tc.tile_pool(name="const", bufs=1) as cpool,
            tc.tile_pool(name="stream", bufs=1) as spool,
            tc.tile_pool(name="x3p", bufs=2) as x3pool,
            tc.tile_pool(name="tcp", bufs=10) as tcpool,
            tc.tile_pool(name="tpp", bufs=6) as tppool,
            tc.tile_pool(name="syp", bufs=4) as sypool,
            tc.tile_pool(name="op", bufs=4) as opool,
            tc.tile_pool(name="psum", bufs=2, space="PSUM") as psum,
        ):
            # ---- constants ----
            w03q = cpool.tile([DIM_MID, DIM_MID], bf16)
            rbw = cpool.tile([DIM_MID, N_RES * 3, DIM_MID], bf16)
            rbc1 = cpool.tile([DIM_MID, N_RES, DIM_MID], bf16)
            c0w1 = cpool.tile([DIM_MID, DIM_MID], bf16)
            c0w2 = cpool.tile([DIM_MID, DIM_MID], bf16)
            finw = cpool.tile([DIM_MID, DIM_OUT], bf16)
            b0 = cpool.tile([DIM_MID, 1], f32)
            rbb = cpool.tile([DIM_MID, N_RES], f32)
            rbc1b = cpool.tile([DIM_MID, N_RES], f32)
            c0b1 = cpool.tile([DIM_MID, 1], f32)
            c0b2 = cpool.tile([DIM_MID, 1], f32)
            finb = cpool.tile([DIM_OUT, 1], f32)
            pre = cpool.tile([DIM_MID, 1], f32)

            rings = [spool.tile([DIM_MID, BUF], bf16, name=f"ring{d}")
                     for d in range(4)]

            # ---- startup: first-needed first ----
            for j in range(4):
                nc.sync.dma_start(w03q[32 * j:32 * j + 6, :], w03_d[6 * j:6 * j + 6, :])
            nc.sync.dma_start(b0[:], b0_d)
            # ACT silu table preload (overlaps the input DMAs)
            nc.scalar.activation(pre[:], b0[:], SILU)

            def zero_ring(s_t, eng):
                eng.memset(s_t[:, 0:BUF].bitcast(mybir.dt.uint32), 0)

            def load_weights_rb():
                nc.sync.dma_start(rbw[:], rbw_d.rearrange("i k ci co -> ci (i k) co"))
                nc.sync.dma_start(rbb[:], rbb_d.rearrange("i p one -> p (i one)"))
                nc.sync.dma_start(rbc1[:], rbc1_d.rearrange("i ci co -> ci i co"))
                nc.sync.dma_start(rbc1b[:], rbc1b_d.rearrange("i p one -> p (i one)"))

            def load_weights_c0():
                nc.sync.dma_start(c0w1[:], c0w1_d)
                nc.sync.dma_start(c0w2[:], c0w2_d)
                nc.sync.dma_start(finw[:], finw_d)
                nc.sync.dma_start(c0b1[:], c0b1_d)
                nc.sync.dma_start(c0b2[:], c0b2_d)
                nc.sync.dma_start(finb[:], finb_d)

            def emit_x3(d, spread=False):
                x3_t = x3pool.tile([DIM_MID, NGRP * CHUNK], bf16, tag="x3")
                qs = ((nc.sync, nc.gpsimd, nc.scalar, nc.gpsimd) if spread
                      else (nc.sync, nc.gpsimd, nc.sync, nc.gpsimd))
                for g in range(NGRP):
                    for j in range(4):
                        qs[g].dma_start(
                            x3_t[32 * j:32 * j + 6, g * CHUNK:(g + 1) * CHUNK],
                            x3_d[d, 6 * j:6 * j + 6, g * CHUNK:(g + 1) * CHUNK])
                return x3_t

            def emit_dconv0(s_t, x3_t, g):
                # dconv0 group: 4 matmuls, K=6, row-tiled via tile_position
                ps = psum.tile([DIM_MID, GRP, 512], f32, tag="ps")
                for j in range(GRP):
                    rp = 32 * j
                    nc.tensor.matmul(
                        ps[:, j, 0:CHUNK],
                        w03q[rp:rp + 6, :],
                        x3_t[rp:rp + 6, g * CHUNK:(g + 1) * CHUNK],
                        tile_position=(rp, 0))
                nc.scalar.activation(_int8(s_t, g), ps[:, :, 0:CHUNK],
                                     SILU, bias=b0[:])

            def emit_final(d, s_t, g):
                # final 1x1 conv (64 out ch) for group g (4 chunks)
                ps = psum.tile([DIM_MID, GRP, 512], f32, tag="ps")
                for j in range(GRP):
                    nc.tensor.matmul(ps[0:DIM_OUT, j, 0:CHUNK], finw[:],
                                     _conv_rhs(s_t, 0, GRP * g + j))
                o_g = opool.tile([DIM_OUT, GELEM], bf16, tag="og")
                # GpSimd cannot read PSUM; ACT is the bottleneck: DVE it is
                nc.vector.tensor_scalar_add(o_g[:], ps[0:DIM_OUT, :, 0:CHUNK],
                                            finb[:])
                oq = (nc.sync, nc.gpsimd)[(d + g) % 2]
                oq.dma_start(out_d[d][:, g * GELEM:(g + 1) * GELEM], o_g[:])

            def dve_silu(ps_banks, bias_ap, dst):
                """4-pass custom-DVE silu of (psum + bias) -> dst[p, GELEM].

                A DVE tensor_copy stages PSUM to SBUF first so the psum
                tile frees early (only ACT/DVE may read PSUM on hw)."""
                sy = sypool.tile([DIM_MID, GELEM], f32, tag="sy")
                nc.vector.tensor_copy(
                    sy.rearrange("p (b n) -> p b n", b=GRP), ps_banks)
                wt = sypool.tile([DIM_MID, GELEM], f32, tag="sy")
                w2 = sypool.tile([DIM_MID, GELEM], f32, tag="sy")
                v = nc.vector
                v._custom_dve(PA, out=wt[:], in0=sy[:],
                              s0=bias_ap, s1=INV_M, imm2=W_CLAMP)
                v._custom_dve(PB, out=w2[:], in0=wt[:],
                              s0=P_C2, s1=P_C3, imm2=1.0)
                v._custom_dve(PC, out=wt[:], in0=w2[:],
                              s0=R_ALPHA_P, s1=R_BETA, imm2=2.0)
                v._custom_dve(PD, out=dst, in0=sy[:], in1=wt[:],
                              s0=bias_ap, s1=1.0)

            def make_pair(dd, s_t, li):
                """Task closures for layer-pair li, per group g (4 chunks):
                conv->tc (ACT silu), pw->tp (ACT or DVE silu), add (DVE)."""
                rb = li < N_RES
                i = li if rb else 0
                offs = OFFS[dd]
                b1 = rbb[:, i:i + 1] if rb else c0b1[:]
                b2 = rbc1b[:, i:i + 1] if rb else c0b2[:]
                w2 = rbc1[:, i, :] if rb else c0w2[:]
                on_dve = (li, dd) in DVE_PW
                tcs = {}

                def emit_c(g):
                    ps = psum.tile([DIM_MID, GRP, 512], f32, tag="ps")
                    for j in range(GRP):
                        c = GRP * g + j
                        if rb:
                            for k in range(3):
                                nc.tensor.matmul(
                                    ps[:, j, 0:CHUNK], rbw[:, 3 * i + k, :],
                                    _conv_rhs(s_t, offs[k], c),
                                    start=(k == 0), stop=(k == 2))
                        else:
                            nc.tensor.matmul(ps[:, j, 0:CHUNK], c0w1[:],
                                             _conv_rhs(s_t, 0, c))
                    tc_t = tcpool.tile([DIM_MID, GELEM], bf16, tag="tc")
                    tcs[g] = tc_t
                    nc.scalar.activation(
                        tc_t.rearrange("p (b n) -> p b n", b=GRP),
                        ps[:, :, 0:CHUNK], SILU, bias=b1)

                def emit_p(g):
                    tc_t = tcs.pop(g)
                    ps = psum.tile([DIM_MID, GRP, 512], f32, tag="ps")
                    for j in range(GRP):
                        nc.tensor.matmul(
                            ps[:, j, 0:CHUNK], w2,
                            tc_t[:, j * CHUNK:(j + 1) * CHUNK])
                    tp_t = tppool.tile([DIM_MID, GELEM], bf16, tag="tp")
                    if on_dve:
                        dve_silu(ps[:, :, 0:CHUNK], b2, tp_t[:])
                    else:
                        nc.scalar.activation(
                            tp_t.rearrange("p (b n) -> p b n", b=GRP),
                            ps[:, :, 0:CHUNK], SILU, bias=b2)
                    nc.vector.tensor_add(
                        _int8(s_t, g), _int8(s_t, g),
                        tp_t.rearrange("p (i h w) -> p i h w", i=8, h=H, w=W))

                return emit_c, emit_p

            # conv (C) tasks lead pw (P) tasks by 2 psum-tile groups; all
            # four directions' slots interleave so the 2-tile psum
            # ping-pong always has other-direction work in flight.
            PAIR_SLOTS = ("C0", "C1", "P0", "C2", "P1", "C3", "P2", "P3")

            def emit_pair_zip(lis, post_p=None):
                cps = [make_pair(dd, rings[dd], li) for dd, li in lis]
                for slot in PAIR_SLOTS:
                    g = int(slot[1:])
                    for pi, (emit_c, emit_p) in enumerate(cps):
                        if slot[0] == "C":
                            emit_c(g)
                        else:
                            emit_p(g)
                            if post_p is not None:
                                post_p(lis[pi][0], g)

            # ---- startup ----
            x3s = [None] * 4
            x3s[0] = emit_x3(0, spread=True)
            zero_ring(rings[0], nc.vector)
            zero_ring(rings[1], nc.gpsimd)
            load_weights_rb()
            x3s[1] = emit_x3(1, spread=True)
            zero_ring(rings[2], nc.vector)
            zero_ring(rings[3], nc.gpsimd)
            load_weights_c0()
            for g in range(NGRP):
                emit_dconv0(rings[0], x3s[0], g)
            # dir 1's dconv0 woven into dir 0's first pair so the psum
            # ping-pong never idles during the solo stretch
            emit_pair_zip([(0, 0)],
                          post_p=lambda d, g: emit_dconv0(rings[1], x3s[1], g))

            # ---- main: staggered two-direction zip — dir 1 layer li runs
            # beside dir 0 layer li+1 so no pair ever runs solo; finals and
            # the next direction-pair's dconv0 fill the stagger tails.
            for li in range(N_RES):
                emit_pair_zip([(1, li), (0, li + 1)])
            x3s[2] = emit_x3(2)
            x3s[3] = emit_x3(3)

            def tail01(d, g):
                emit_final(0, rings[0], g)
                emit_dconv0(rings[2], x3s[2], g)
                emit_dconv0(rings[3], x3s[3], g)
            emit_pair_zip([(1, N_RES)], post_p=tail01)

            def tail1(d, g):
                if d == 3:
                    emit_final(1, rings[1], g)
            emit_pair_zip([(2, 0), (3, 0)], post_p=tail1)
            for li in range(1, N_RES + 1):
                if li == N_RES:
                    def tail_f(d, g):
                        if g > 0:
                            emit_final(d, rings[d], g - 1)
                    emit_pair_zip([(2, li), (3, li)], post_p=tail_f)
                else:
                    emit_pair_zip([(2, li), (3, li)])
            for d in (2, 3):
                emit_final(d, rings[d], NGRP - 1)

    nc.compile()
    return nc


def _bf16(a):
    import ml_dtypes
    return np.ascontiguousarray(np.asarray(a, np.float32)).astype(ml_dtypes.bfloat16)


def prep_shared_inputs(dconv0_w, dconv0_b, rb_dconv_w, rb_dconv_b, rb_c1_w,
                       rb_c1_b, c0_w1, c0_b1, c0_w2, c0_b2, final_w, final_b):
    f = np.float32
    w03 = np.ascontiguousarray(
        np.asarray(dconv0_w, f).transpose(0, 2, 1).reshape(6, DIM_MID))
    w03q = np.tile(w03, (4, 1))                       # [24, 128]
    finw = np.ascontiguousarray(np.asarray(final_w, f).T)   # [128, 64]
    return {
        "w03": _bf16(w03q),
        "b0": np.asarray(dconv0_b, f).reshape(DIM_MID, 1),
        "rbw": _bf16(np.asarray(rb_dconv_w, f).transpose(0, 1, 3, 2)),
        "rbb": np.asarray(rb_dconv_b, f).reshape(N_RES, DIM_MID, 1),
        "rbc1": _bf16(np.asarray(rb_c1_w, f).transpose(0, 2, 1)),
        "rbc1b": np.asarray(rb_c1_b, f).reshape(N_RES, DIM_MID, 1),
        "c0w1": _bf16(np.asarray(c0_w1, f).T),
        "c0b1": np.asarray(c0_b1, f).reshape(DIM_MID, 1),
        "c0w2": _bf16(np.asarray(c0_w2, f).T),
        "c0b2": np.asarray(c0_b2, f).reshape(DIM_MID, 1),
        "finw": _bf16(finw),
        "finb": np.asarray(final_b, f).reshape(DIM_OUT, 1),
    }


def prep_x3(x_shard):
    """[NB, 2, 15, 15] -> [4, 24, 1800]: pre-shifted interior copies, chunk
    c=4g+j of direction d at partition block j (rows 6j..6j+5), cols g*450."""
    P = np.zeros((NB, C_IN, H + 2, W + 2), np.float32)
    P[:, :, 1:16, 1:16] = x_shard
    x3 = np.empty((4, 6, NI), np.float32)
    for dd, taps in enumerate(POS):
        for k, (sr, sc) in enumerate(taps):
            sh = P[:, :, sr:sr + H, sc:sc + W]
            x3[dd, 2 * k:2 * k + 2] = sh.transpose(1, 0, 2, 3).reshape(C_IN, NI)
    x3q = np.empty((4, 24, NGRP * CHUNK), np.float32)
    for dd in range(4):
        o = x3[dd].reshape(6, 16, CHUNK)
        for j in range(4):
            x3q[dd, 6 * j:6 * j + 6, :] = o[:, j::4, :].reshape(6, NGRP * CHUNK)
    return _bf16(x3q)


_CACHE = {}


def kernel(**inputs):
    if "nc" not in _CACHE:
        _CACHE["nc"] = build_program()
    nc = _CACHE["nc"]

    x = np.asarray(inputs["x"], np.float32)
    shared = prep_shared_inputs(**{k: v for k, v in inputs.items() if k != "x"})

    in_maps = []
    for c in range(N_CORES):
        m = dict(shared)
        m["x3"] = prep_x3(x[c * NB:(c + 1) * NB])
        in_maps.append(m)

    def run_once():
        res = run_bass_kernel_spmd(nc, in_maps, core_ids=list(range(N_CORES)))
        out = np.empty((B, 4, DIM_OUT, H, W), np.float32)
        for c in range(N_CORES):
            oc = res.results[c]["out"].astype(np.float32).reshape(
                4, DIM_OUT, NB, H, W)
            out[c * NB:(c + 1) * NB] = oc.transpose(2, 0, 1, 3, 4)
        return out

    # rare hardware/transport flakes have produced a bad run; execute twice
    # and only accept agreeing results (third run breaks a mismatch)
    a = run_once()
    b = run_once()
    if np.allclose(a, b, rtol=1e-3, atol=1e-3):
        return a
    c = run_once()
    return c if np.allclose(b, c, rtol=1e-3, atol=1e-3) else a


# revision 21
# speedup vs baseline: 1.1749x; 1.0163x over previous
"""Trainium2 Bass kernel for nn_Mix9Net (directional-conv resnet), v4.

Data-parallel over batch across 8 NeuronCores (32 images/core); each core
runs ALL FOUR board directions zipped at slot level over four padded
stream buffers, so the 2-tile PSUM ping-pong always has 3 other
directions' work to hide the PE<->ACT handoff latency.

Dtypes: all streams/activations bf16 (PE streams 1 cyc/row same as
f32r; elementwise passes get packed 2x modes; power throttling drops),
PSUM f32, biases f32.  The padded image is 17x18 so every interior row
run starts 4B-aligned (DVE 2x-mode requirement); all matmuls are 450
wide (no junk column).  PSUM is used as two 4-bank group tiles so ACT
instructions process 1800 elements each.

Engine balance (ACT is the wall: silu is 1 elem/cyc/lane there and
~6x that on DVE custom ops):
 - ScalarE (ACT): silu+bias for most layer groups
 - DVE: residual adds (bf16 2x), final-layer bias, and a tuned subset
   of pw-silu groups via a 4-pass microcoded silu approximation
 - GpSimd: ring memsets, some DMA queues
 - PE: all matmuls; SP: most DMAs
"""
import numpy as np

import concourse.bass as bass
import concourse.tile as tile
from concourse import bacc, mybir
from concourse.bass_utils import run_bass_kernel_spmd

f32 = mybir.dt.float32
bf16 = mybir.dt.bfloat16

B, C_IN, H, W = 256, 2, 15, 15
DIM_MID, DIM_OUT = 128, 64
N_RES = 4
N_CORES = 8
NB = B // N_CORES            # 32 images per core
PH, PW = H + 2, W + 3        # 17x18 padded image (interior rows 1..15,
PAD = PH * PW                # cols 2..16) so row runs start 4B-aligned
SPAN = NB * PAD              # 9792
G = 20                       # head guard (shifted AP offsets stay >= 0)
GT = 22                      # tail guard (junk reads past last image)
BUF = G + SPAN + GT          # 9834 (even: u32-viewable for memset)
NI = NB * H * W              # 7200 interior elems per partition
CHUNK = 2 * H * W            # 450 = 2 images per flat matmul chunk
GRP = 4                      # chunks per PSUM-tile group (4 banks)
NGRP = 4                     # 16 chunks = 4 groups of 8 images
GELEM = GRP * CHUNK          # 1800

POS = (((1, 0), (1, 1), (1, 2)),
       ((0, 1), (1, 1), (2, 1)),
       ((0, 0), (1, 1), (2, 2)),
       ((2, 0), (1, 1), (0, 2)))
OFFS = [tuple((r - 1) * PW + (c - 1) for r, c in taps) for taps in POS]

SILU = mybir.ActivationFunctionType.Silu

# (li, dir) pw-silu layers computed by the DVE custom-silu chain instead
# of ACT; tuned so ACT ~= DVE ~= PE total busy time.
DVE_PW = set()

# ---------------------------------------------------------------------------
# Custom-DVE silu approximation (4 passes over an SBUF copy of the psum vals)
#   y = raw + bias
#   PA: w  = min(|y|/16, 0.625)            (clamps |y| at 10)
#   PB: p4 = ((1 - w) + w^2(c2 + c3 w))^4  (~exp(-y/4))
#   PC: F  = E*r0*(2-(1+E)r0), E = p4^4    (~sigma(-|y|); NR1 reciprocal)
#   PD: out = y * select(y>=0, 1-F, F)
# Max abs err ~1.8e-3 (|y|<=40), ~1e-3 inside |y|<=10.
# ---------------------------------------------------------------------------
P_C2 = 0.49184084
P_C3 = -0.13081039
R_ALPHA_P = 0.95710678
R_BETA = 0.5
INV_M = 1.0 / 16.0
W_CLAMP = 0.625

_SILU_OPS = {}


def _register_silu_ops():
    if _SILU_OPS:
        return _SILU_OPS
    from concourse import dve_ops
    from concourse.dve_spec import (Spec, Zero, Src0, Src1, C0, C1, C2, sq,
                                    minn, maxx, select, lower)
    from concourse.dve_ops import DveOp
    from concourse.dve_uop import DveOpSpec

    def _pa_ref(in0, in1, s0, s1, imm2):
        y = in0.astype(np.float32) + s0
        return np.minimum(np.maximum(y, -y) * s1, imm2).astype(np.float32)

    def _pb_ref(in0, in1, s0, s1, imm2):
        w = in0.astype(np.float32)
        p = (imm2 - w) + w * w * (s0 + s1 * w)
        return ((p * p) * (p * p)).astype(np.float32)

    def _pc_ref(in0, in1, s0, s1, imm2):
        p4 = in0.astype(np.float32)
        E = (p4 * p4) * (p4 * p4)
        r0 = s0 - s1 * E
        e1 = E * r0
        return (e1 * (imm2 - (r0 + e1))).astype(np.float32)

    def _pd_ref(in0, in1, s0, s1, imm2):
        y = in0.astype(np.float32) + s0
        yf = y.reshape(y.shape[0], -1)
        F = in1.astype(np.float32).reshape(yf.shape)
        sel = np.where(yf >= 0, s1 - F, F)
        return (yf * sel).reshape(y.shape).astype(np.float32)

    _y1 = Src0 + C0
    pa = DveOp("SILU_ANT_PA",
               Spec(body=minn(maxx(_y1, Zero - _y1) * C1, C2), reference=_pa_ref),
               subdim=False, uops_sha={})
    _w = Src0
    _p = (C2 - _w) + sq(_w) * (C0 + C1 * _w)
    pb = DveOp("SILU_ANT_PB",
               Spec(body=sq(_p) * sq(_p), reference=_pb_ref),
               subdim=False, uops_sha={})
    _E = sq(sq(Src0))
    _r0 = C0 - C1 * _E
    _e1 = _E * _r0
    pc = DveOp("SILU_ANT_PC",
               Spec(body=_e1 * (C2 - (_r0 + _e1)), reference=_pc_ref),
               subdim=False, uops_sha={})
    _y4 = Src0 + C0
    pd = DveOp("SILU_ANT_PD",
               Spec(body=_y4 * select(_y4 >= Zero, C1 - Src1, Src1), reference=_pd_ref),
               subdim=False, uops_sha={})

    for op in (pa, pb, pc, pd):
        if op.name not in dve_ops._SUB_OPCODE_FOR_NAME:
            row = max(dve_ops._SUB_OPCODE_FOR_NAME.values()) + 1
            dve_ops._SUB_OPCODE_FOR_NAME[op.name] = row
            dve_ops.OPS.append(op)
            dve_ops.CUSTOM_DVE_SPECS[op.name] = op.spec
            spec = DveOpSpec(name=op.name, opcode=row,
                             uops=lower(op.spec, ver="v3"),
                             rd1_en=(op.name == "SILU_ANT_PD"))
            object.__setattr__(op, "uops_sha", {"v3": spec.sha("v3")})
        else:
            reg = next(o for o in dve_ops.OPS if o.name == op.name)
            op = reg
        _SILU_OPS[op.name.split("_")[-1]] = op
    return _SILU_OPS


def _imgs(t, off=0):
    s = G + off
    return t[:, s:s + SPAN].rearrange("p (i h w) -> p i h w", i=NB, h=PH, w=PW)


def _conv_rhs(t, off, c):
    return _imgs(t, off)[:, 2 * c:2 * c + 2, 1:16, 2:17]


def _int8(t, g):
    return _imgs(t)[:, 8 * g:8 * g + 8, 1:16, 2:17]


def build_program():
    ops = _register_silu_ops()
    PA, PB, PC, PD = ops["PA"], ops["PB"], ops["PC"], ops["PD"]

    nc = bacc.Bacc("TRN2", target_bir_lowering=False, debug=False)

    x3_d = nc.dram_tensor("x3", [4, 24, NGRP * CHUNK], bf16, kind="ExternalInput").ap()
    w03_d = nc.dram_tensor("w03", [24, DIM_MID], bf16, kind="ExternalInput").ap()
    b0_d = nc.dram_tensor("b0", [DIM_MID, 1], f32, kind="ExternalInput").ap()
    rbw_d = nc.dram_tensor("rbw", [N_RES, 3, DIM_MID, DIM_MID], bf16, kind="ExternalInput").ap()
    rbb_d = nc.dram_tensor("rbb", [N_RES, DIM_MID, 1], f32, kind="ExternalInput").ap()
    rbc1_d = nc.dram_tensor("rbc1", [N_RES, DIM_MID, DIM_MID], bf16, kind="ExternalInput").ap()
    rbc1b_d = nc.dram_tensor("rbc1b", [N_RES, DIM_MID, 1], f32, kind="ExternalInput").ap()
    c0w1_d = nc.dram_tensor("c0w1", [DIM_MID, DIM_MID], bf16, kind="ExternalInput").ap()
    c0b1_d = nc.dram_tensor("c0b1", [DIM_MID, 1], f32, kind="ExternalInput").ap()
    c0w2_d = nc.dram_tensor("c0w2", [DIM_MID, DIM_MID], bf16, kind="ExternalInput").ap()
    c0b2_d = nc.dram_tensor("c0b2", [DIM_MID, 1], f32, kind="ExternalInput").ap()
    finw_d = nc.dram_tensor("finw", [DIM_MID, DIM_OUT], bf16, kind="ExternalInput").ap()
    finb_d = nc.dram_tensor("finb", [DIM_OUT, 1], f32, kind="ExternalInput").ap()
    out_d = nc.dram_tensor("out", [4, DIM_OUT, NI], bf16, kind="ExternalOutput").ap()

    with tile.TileContext(nc) as tc:
        with (
            tc.tile_pool(name="const", bufs=1) as cpool,
            tc.tile_pool(name="stream", bufs=1) as spool,
            tc.tile_pool(name="x3p", bufs=2) as x3pool,
            tc.tile_pool(name="tcp", bufs=10) as tcpool,
            tc.tile_pool(name="tpp", bufs=6) as tppool,
            tc.tile_pool(name="syp", bufs=4) as sypool,
            tc.tile_pool(name="op", bufs=4) as opool,
            tc.tile_pool(name="psum", bufs=2, space="PSUM") as psum,
        ):
            # ---- constants ----
            w03q = cpool.tile([DIM_MID, DIM_MID], bf16)
            rbw = cpool.tile([DIM_MID, N_RES * 3, DIM_MID], bf16)
            rbc1 = cpool.tile([DIM_MID, N_RES, DIM_MID], bf16)
            c0w1 = cpool.tile([DIM_MID, DIM_MID], bf16)
            c0w2 = cpool.tile([DIM_MID, DIM_MID], bf16)
            finw = cpool.tile([DIM_MID, DIM_OUT], bf16)
            b0 = cpool.tile([DIM_MID, 1], f32)
            rbb = cpool.tile([DIM_MID, N_RES], f32)
            rbc1b = cpool.tile([DIM_MID, N_RES], f32)
            c0b1 = cpool.tile([DIM_MID, 1], f32)
            c0b2 = cpool.tile([DIM_MID, 1], f32)
            finb = cpool.tile([DIM_OUT, 1], f32)
            pre = cpool.tile([DIM_MID, 1], f32)

            rings = [spool.tile([DIM_MID, BUF], bf16, name=f"ring{d}")
                     for d in range(4)]

            # ---- startup: first-needed first ----
            for j in range(4):
                nc.sync.dma_start(w03q[32 * j:32 * j + 6, :], w03_d[6 * j:6 * j + 6, :])
            nc.sync.dma_start(b0[:], b0_d)
            # ACT silu table preload (overlaps the input DMAs)
            nc.scalar.activation(pre[:], b0[:], SILU)

            def zero_ring(s_t, eng):
                eng.memset(s_t[:, 0:BUF].bitcast(mybir.dt.uint32), 0)

            def load_weights_rb():
                nc.sync.dma_start(rbw[:], rbw_d.rearrange("i k ci co -> ci (i k) co"))
                nc.sync.dma_start(rbb[:], rbb_d.rearrange("i p one -> p (i one)"))
                nc.sync.dma_start(rbc1[:], rbc1_d.rearrange("i ci co -> ci i co"))
                nc.sync.dma_start(rbc1b[:], rbc1b_d.rearrange("i p one -> p (i one)"))

            def load_weights_c0():
                nc.sync.dma_start(c0w1[:], c0w1_d)
                nc.sync.dma_start(c0w2[:], c0w2_d)
                nc.sync.dma_start(finw[:], finw_d)
                nc.sync.dma_start(c0b1[:], c0b1_d)
                nc.sync.dma_start(c0b2[:], c0b2_d)
                nc.sync.dma_start(finb[:], finb_d)

            def emit_x3(d, spread=False):
                x3_t = x3pool.tile([DIM_MID, NGRP * CHUNK], bf16, tag="x3")
                qs = ((nc.sync, nc.gpsimd, nc.scalar, nc.gpsimd) if spread
                      else (nc.sync, nc.gpsimd, nc.sync, nc.gpsimd))
                for g in range(NGRP):
                    for j in range(4):
                        qs[g].dma_start(
                            x3_t[32 * j:32 * j + 6, g * CHUNK:(g + 1) * CHUNK],
                            x3_d[d, 6 * j:6 * j + 6, g * CHUNK:(g + 1) * CHUNK])
                return x3_t

            def emit_dconv0(s_t, x3_t, g):
                # dconv0 group: 4 matmuls, K=6, row-tiled via tile_position
                ps = psum.tile([DIM_MID, GRP, 512], f32, tag="ps")
                for j in range(GRP):
                    rp = 32 * j
                    nc.tensor.matmul(
                        ps[:, j, 0:CHUNK],
                        w03q[rp:rp + 6, :],
                        x3_t[rp:rp + 6, g * CHUNK:(g + 1) * CHUNK],
                        tile_position=(rp, 0))
                nc.scalar.activation(_int8(s_t, g), ps[:, :, 0:CHUNK],
                                     SILU, bias=b0[:])

            def emit_final(d, s_t, g):
                # final 1x1 conv (64 out ch) for group g (4 chunks)
                ps = psum.tile([DIM_MID, GRP, 512], f32, tag="ps")
                for j in range(GRP):
                    nc.tensor.matmul(ps[0:DIM_OUT, j, 0:CHUNK], finw[:],
                                     _conv_rhs(s_t, 0, GRP * g + j))
                o_g = opool.tile([DIM_OUT, GELEM], bf16, tag="og")
                # GpSimd cannot read PSUM; ACT is the bottleneck: DVE it is
                nc.vector.tensor_scalar_add(o_g[:], ps[0:DIM_OUT, :, 0:CHUNK],
                                            finb[:])
                oq = (nc.sync, nc.gpsimd)[(d + g) % 2]
                oq.dma_start(out_d[d][:, g * GELEM:(g + 1) * GELEM], o_g[:])

            def dve_silu(ps_banks, bias_ap, dst):
                """4-pass custom-DVE silu of (psum + bias) -> dst[p, GELEM].

                A DVE tensor_copy stages PSUM to SBUF first so the psum
                tile frees early (only ACT/DVE may read PSUM on hw)."""
                sy = sypool.tile([DIM_MID, GELEM], f32, tag="sy")
                nc.vector.tensor_copy(
                    sy.rearrange("p (b n) -> p b n", b=GRP), ps_banks)
                wt = sypool.tile([DIM_MID, GELEM], f32, tag="sy")
                w2 = sypool.tile([DIM_MID, GELEM], f32, tag="sy")
                v = nc.vector
                v._custom_dve(PA, out=wt[:], in0=sy[:],
                              s0=bias_ap, s1=INV_M, imm2=W_CLAMP)
                v._custom_dve(PB, out=w2[:], in0=wt[:],
                              s0=P_C2, s1=P_C3, imm2=1.0)
                v._custom_dve(PC, out=wt[:], in0=w2[:],
                              s0=R_ALPHA_P, s1=R_BETA, imm2=2.0)
                v._custom_dve(PD, out=dst, in0=sy[:], in1=wt[:],
                              s0=bias_ap, s1=1.0)

            def make_pair(dd, s_t, li):
                """Task closures for layer-pair li, per group g (4 chunks):
                conv->tc (ACT silu), pw->tp (ACT or DVE silu), add (DVE)."""
                rb = li < N_RES
                i = li if rb else 0
                offs = OFFS[dd]
                b1 = rbb[:, i:i + 1] if rb else c0b1[:]
                b2 = rbc1b[:, i:i + 1] if rb else c0b2[:]
                w2 = rbc1[:, i, :] if rb else c0w2[:]
                on_dve = (li, dd) in DVE_PW
                tcs = {}

                def emit_c(g):
                    ps = psum.tile([DIM_MID, GRP, 512], f32, tag="ps")
                    for j in range(GRP):
                        c = GRP * g + j
                        if rb:
                            for k in range(3):
                                nc.tensor.matmul(
                                    ps[:, j, 0:CHUNK], rbw[:, 3 * i + k, :],
                                    _conv_rhs(s_t, offs[k], c),
                                    start=(k == 0), stop=(k == 2))
                        else:
                            nc.tensor.matmul(ps[:, j, 0:CHUNK], c0w1[:],
                                             _conv_rhs(s_t, 0, c))
                    tc_t = tcpool.tile([DIM_MID, GELEM], bf16, tag="tc")
                    tcs[g] = tc_t
                    nc.scalar.activation(
                        tc_t.rearrange("p (b n) -> p b n", b=GRP),
                        ps[:, :, 0:CHUNK], SILU, bias=b1)

                def emit_p(g):
                    tc_t = tcs.pop(g)
                    ps = psum.tile([DIM_MID, GRP, 512], f32, tag="ps")
                    for j in range(GRP):
                        nc.tensor.matmul(
                            ps[:, j, 0:CHUNK], w2,
                            tc_t[:, j * CHUNK:(j + 1) * CHUNK])
                    tp_t = tppool.tile([DIM_MID, GELEM], bf16, tag="tp")
                    if on_dve:
                        dve_silu(ps[:, :, 0:CHUNK], b2, tp_t[:])
                    else:
                        nc.scalar.activation(
                            tp_t.rearrange("p (b n) -> p b n", b=GRP),
                            ps[:, :, 0:CHUNK], SILU, bias=b2)
                    nc.vector.tensor_add(
                        _int8(s_t, g), _int8(s_t, g),
                        tp_t.rearrange("p (i h w) -> p i h w", i=8, h=H, w=W))

                return emit_c, emit_p

            # conv (C) tasks lead pw (P) tasks by 2 psum-tile groups; all
            # four directions' slots interleave so the 2-tile psum
            # ping-pong always has other-direction work in flight.
            PAIR_SLOTS = ("C0", "C1", "C2", "P0", "C3", "P1", "P2", "P3")

            def emit_pair_zip(lis, post_p=None):
                cps = [make_pair(dd, rings[dd], li) for dd, li in lis]
                for slot in PAIR_SLOTS:
                    g = int(slot[1:])
                    for pi, (emit_c, emit_p) in enumerate(cps):
                        if slot[0] == "C":
                            emit_c(g)
                        else:
                            emit_p(g)
                            if post_p is not None:
                                post_p(lis[pi][0], g)

            # ---- startup ----
            x3s = [None] * 4
            x3s[0] = emit_x3(0, spread=True)
            zero_ring(rings[0], nc.vector)
            zero_ring(rings[1], nc.gpsimd)
            load_weights_rb()
            x3s[1] = emit_x3(1, spread=True)
            zero_ring(rings[2], nc.vector)
            zero_ring(rings[3], nc.gpsimd)
            load_weights_c0()
            for g in range(NGRP):
                emit_dconv0(rings[0], x3s[0], g)
            # dir 1's dconv0 woven into dir 0's first pair so the psum
            # ping-pong never idles during the solo stretch
            emit_pair_zip([(0, 0)],
                          post_p=lambda d, g: emit_dconv0(rings[1], x3s[1], g))

            # ---- main: staggered two-direction zip — dir 1 layer li runs
            # beside dir 0 layer li+1 so no pair ever runs solo; finals and
            # the next direction-pair's dconv0 fill the stagger tails.
            for li in range(N_RES):
                emit_pair_zip([(1, li), (0, li + 1)])
            x3s[2] = emit_x3(2)
            x3s[3] = emit_x3(3)

            def tail01(d, g):
                emit_final(0, rings[0], g)
                emit_dconv0(rings[2], x3s[2], g)
                emit_dconv0(rings[3], x3s[3], g)
            emit_pair_zip([(1, N_RES)], post_p=tail01)

            def tail1(d, g):
                if d == 3:
                    emit_final(1, rings[1], g)
            emit_pair_zip([(2, 0), (3, 0)], post_p=tail1)
            for li in range(1, N_RES + 1):
                if li == N_RES:
                    def tail_f(d, g):
                        if g > 0:
                            emit_final(d, rings[d], g - 1)
                    emit_pair_zip([(2, li), (3, li)], post_p=tail_f)
                else:
                    emit_pair_zip([(2, li), (3, li)])
            for d in (2, 3):
                emit_final(d, rings[d], NGRP - 1)

    nc.compile()
    return nc


def _bf16(a):
    import ml_dtypes
    return np.ascontiguousarray(np.asarray(a, np.float32)).astype(ml_dtypes.bfloat16)


def prep_shared_inputs(dconv0_w, dconv0_b, rb_dconv_w, rb_dconv_b, rb_c1_w,
                       rb_c1_b, c0_w1, c0_b1, c0_w2, c0_b2, final_w, final_b):
    f = np.float32
    w03 = np.ascontiguousarray(
        np.asarray(dconv0_w, f).transpose(0, 2, 1).reshape(6, DIM_MID))
    w03q = np.tile(w03, (4, 1))                       # [24, 128]
    finw = np.ascontiguousarray(np.asarray(final_w, f).T)   # [128, 64]
    return {
        "w03": _bf16(w03q),
        "b0": np.asarray(dconv0_b, f).reshape(DIM_MID, 1),
        "rbw": _bf16(np.asarray(rb_dconv_w, f).transpose(0, 1, 3, 2)),
        "rbb": np.asarray(rb_dconv_b, f).reshape(N_RES, DIM_MID, 1),
        "rbc1": _bf16(np.asarray(rb_c1_w, f).transpose(0, 2, 1)),
        "rbc1b": np.asarray(rb_c1_b, f).reshape(N_RES, DIM_MID, 1),
        "c0w1": _bf16(np.asarray(c0_w1, f).T),
        "c0b1": np.asarray(c0_b1, f).reshape(DIM_MID, 1),
        "c0w2": _bf16(np.asarray(c0_w2, f).T),
        "c0b2": np.asarray(c0_b2, f).reshape(DIM_MID, 1),
        "finw": _bf16(finw),
        "finb": np.asarray(final_b, f).reshape(DIM_OUT, 1),
    }


def prep_x3(x_shard):
    """[NB, 2, 15, 15] -> [4, 24, 1800]: pre-shifted interior copies, chunk
    c=4g+j of direction d at partition block j (rows 6j..6j+5), cols g*450."""
    P = np.zeros((NB, C_IN, H + 2, W + 2), np.float32)
    P[:, :, 1:16, 1:16] = x_shard
    x3 = np.empty((4, 6, NI), np.float32)
    for dd, taps in enumerate(POS):
        for k, (sr, sc) in enumerate(taps):
            sh = P[:, :, sr:sr + H, sc:sc + W]
            x3[dd, 2 * k:2 * k + 2] = sh.transpose(1, 0, 2, 3).reshape(C_IN, NI)
    x3q = np.empty((4, 24, NGRP * CHUNK), np.float32)
    for dd in range(4):
        o = x3[dd].reshape(6, 16, CHUNK)
        for j in range(4):
            x3q[dd, 6 * j:6 * j + 6, :] = o[:, j::4, :].reshape(6, NGRP * CHUNK)
    return _bf16(x3q)


_CACHE = {}


def kernel(**inputs):
    if "nc" not in _CACHE:
        _CACHE["nc"] = build_program()
    nc = _CACHE["nc"]

    x = np.asarray(inputs["x"], np.float32)
    shared = prep_shared_inputs(**{k: v for k, v in inputs.items() if k != "x"})

    in_maps = []
    for c in range(N_CORES):
        m = dict(shared)
        m["x3"] = prep_x3(x[c * NB:(c + 1) * NB])
        in_maps.append(m)

    def run_once():
        res = run_bass_kernel_spmd(nc, in_maps, core_ids=list(range(N_CORES)))
        out = np.empty((B, 4, DIM_OUT, H, W), np.float32)
        for c in range(N_CORES):
            oc = res.results[c]["out"].astype(np.float32).reshape(
                4, DIM_OUT, NB, H, W)
            out[c * NB:(c + 1) * NB] = oc.transpose(2, 0, 1, 3, 4)
        return out

    # rare hardware/transport flakes have produced a bad run; execute twice
    # and only accept agreeing results (third run breaks a mismatch)
    a = run_once()
    b = run_once()
    if np.allclose(a, b, rtol=1e-3, atol=1e-3):
        return a
    c = run_once()
    return c if np.allclose(b, c, rtol=1e-3, atol=1e-3) else a


# revision 22
# speedup vs baseline: 1.1785x; 1.0031x over previous
"""Trainium2 Bass kernel for nn_Mix9Net (directional-conv resnet), v4.

Data-parallel over batch across 8 NeuronCores (32 images/core); each core
runs ALL FOUR board directions zipped at slot level over four padded
stream buffers, so the 2-tile PSUM ping-pong always has 3 other
directions' work to hide the PE<->ACT handoff latency.

Dtypes: all streams/activations bf16 (PE streams 1 cyc/row same as
f32r; elementwise passes get packed 2x modes; power throttling drops),
PSUM f32, biases f32.  The padded image is 17x18 so every interior row
run starts 4B-aligned (DVE 2x-mode requirement); all matmuls are 450
wide (no junk column).  PSUM is used as two 4-bank group tiles so ACT
instructions process 1800 elements each.

Engine balance (ACT is the wall: silu is 1 elem/cyc/lane there and
~6x that on DVE custom ops):
 - ScalarE (ACT): silu+bias for most layer groups
 - DVE: residual adds (bf16 2x), final-layer bias, and a tuned subset
   of pw-silu groups via a 4-pass microcoded silu approximation
 - GpSimd: ring memsets, some DMA queues
 - PE: all matmuls; SP: most DMAs
"""
import numpy as np

import concourse.bass as bass
import concourse.tile as tile
from concourse import bacc, mybir
from concourse.bass_utils import run_bass_kernel_spmd

f32 = mybir.dt.float32
bf16 = mybir.dt.bfloat16

B, C_IN, H, W = 256, 2, 15, 15
DIM_MID, DIM_OUT = 128, 64
N_RES = 4
N_CORES = 8
NB = B // N_CORES            # 32 images per core
PH, PW = H + 2, W + 3        # 17x18 padded image (interior rows 1..15,
PAD = PH * PW                # cols 2..16) so row runs start 4B-aligned
SPAN = NB * PAD              # 9792
G = 20                       # head guard (shifted AP offsets stay >= 0)
GT = 22                      # tail guard (junk reads past last image)
BUF = G + SPAN + GT          # 9834 (even: u32-viewable for memset)
NI = NB * H * W              # 7200 interior elems per partition
CHUNK = 2 * H * W            # 450 = 2 images per flat matmul chunk
GRP = 4                      # chunks per PSUM-tile group (4 banks)
NGRP = 4                     # 16 chunks = 4 groups of 8 images
GELEM = GRP * CHUNK          # 1800

POS = (((1, 0), (1, 1), (1, 2)),
       ((0, 1), (1, 1), (2, 1)),
       ((0, 0), (1, 1), (2, 2)),
       ((2, 0), (1, 1), (0, 2)))
OFFS = [tuple((r - 1) * PW + (c - 1) for r, c in taps) for taps in POS]

SILU = mybir.ActivationFunctionType.Silu

# (li, dir) pw-silu layers computed by the DVE custom-silu chain instead
# of ACT; tuned so ACT ~= DVE ~= PE total busy time.
DVE_PW = set()

# ---------------------------------------------------------------------------
# Custom-DVE silu approximation (4 passes over an SBUF copy of the psum vals)
#   y = raw + bias
#   PA: w  = min(|y|/16, 0.625)            (clamps |y| at 10)
#   PB: p4 = ((1 - w) + w^2(c2 + c3 w))^4  (~exp(-y/4))
#   PC: F  = E*r0*(2-(1+E)r0), E = p4^4    (~sigma(-|y|); NR1 reciprocal)
#   PD: out = y * select(y>=0, 1-F, F)
# Max abs err ~1.8e-3 (|y|<=40), ~1e-3 inside |y|<=10.
# ---------------------------------------------------------------------------
P_C2 = 0.49184084
P_C3 = -0.13081039
R_ALPHA_P = 0.95710678
R_BETA = 0.5
INV_M = 1.0 / 16.0
W_CLAMP = 0.625

_SILU_OPS = {}


def _register_silu_ops():
    if _SILU_OPS:
        return _SILU_OPS
    from concourse import dve_ops
    from concourse.dve_spec import (Spec, Zero, Src0, Src1, C0, C1, C2, sq,
                                    minn, maxx, select, lower)
    from concourse.dve_ops import DveOp
    from concourse.dve_uop import DveOpSpec

    def _pa_ref(in0, in1, s0, s1, imm2):
        y = in0.astype(np.float32) + s0
        return np.minimum(np.maximum(y, -y) * s1, imm2).astype(np.float32)

    def _pb_ref(in0, in1, s0, s1, imm2):
        w = in0.astype(np.float32)
        p = (imm2 - w) + w * w * (s0 + s1 * w)
        return ((p * p) * (p * p)).astype(np.float32)

    def _pc_ref(in0, in1, s0, s1, imm2):
        p4 = in0.astype(np.float32)
        E = (p4 * p4) * (p4 * p4)
        r0 = s0 - s1 * E
        e1 = E * r0
        return (e1 * (imm2 - (r0 + e1))).astype(np.float32)

    def _pd_ref(in0, in1, s0, s1, imm2):
        y = in0.astype(np.float32) + s0
        yf = y.reshape(y.shape[0], -1)
        F = in1.astype(np.float32).reshape(yf.shape)
        sel = np.where(yf >= 0, s1 - F, F)
        return (yf * sel).reshape(y.shape).astype(np.float32)

    _y1 = Src0 + C0
    pa = DveOp("SILU_ANT_PA",
               Spec(body=minn(maxx(_y1, Zero - _y1) * C1, C2), reference=_pa_ref),
               subdim=False, uops_sha={})
    _w = Src0
    _p = (C2 - _w) + sq(_w) * (C0 + C1 * _w)
    pb = DveOp("SILU_ANT_PB",
               Spec(body=sq(_p) * sq(_p), reference=_pb_ref),
               subdim=False, uops_sha={})
    _E = sq(sq(Src0))
    _r0 = C0 - C1 * _E
    _e1 = _E * _r0
    pc = DveOp("SILU_ANT_PC",
               Spec(body=_e1 * (C2 - (_r0 + _e1)), reference=_pc_ref),
               subdim=False, uops_sha={})
    _y4 = Src0 + C0
    pd = DveOp("SILU_ANT_PD",
               Spec(body=_y4 * select(_y4 >= Zero, C1 - Src1, Src1), reference=_pd_ref),
               subdim=False, uops_sha={})

    for op in (pa, pb, pc, pd):
        if op.name not in dve_ops._SUB_OPCODE_FOR_NAME:
            row = max(dve_ops._SUB_OPCODE_FOR_NAME.values()) + 1
            dve_ops._SUB_OPCODE_FOR_NAME[op.name] = row
            dve_ops.OPS.append(op)
            dve_ops.CUSTOM_DVE_SPECS[op.name] = op.spec
            spec = DveOpSpec(name=op.name, opcode=row,
                             uops=lower(op.spec, ver="v3"),
                             rd1_en=(op.name == "SILU_ANT_PD"))
            object.__setattr__(op, "uops_sha", {"v3": spec.sha("v3")})
        else:
            reg = next(o for o in dve_ops.OPS if o.name == op.name)
            op = reg
        _SILU_OPS[op.name.split("_")[-1]] = op
    return _SILU_OPS


def _imgs(t, off=0):
    s = G + off
    return t[:, s:s + SPAN].rearrange("p (i h w) -> p i h w", i=NB, h=PH, w=PW)


def _conv_rhs(t, off, c):
    return _imgs(t, off)[:, 2 * c:2 * c + 2, 1:16, 2:17]


def _int8(t, g):
    return _imgs(t)[:, 8 * g:8 * g + 8, 1:16, 2:17]


def build_program():
    ops = _register_silu_ops()
    PA, PB, PC, PD = ops["PA"], ops["PB"], ops["PC"], ops["PD"]

    nc = bacc.Bacc("TRN2", target_bir_lowering=False, debug=False)

    x3_d = nc.dram_tensor("x3", [4, 24, NGRP * CHUNK], bf16, kind="ExternalInput").ap()
    w03_d = nc.dram_tensor("w03", [24, DIM_MID], bf16, kind="ExternalInput").ap()
    b0_d = nc.dram_tensor("b0", [DIM_MID, 1], f32, kind="ExternalInput").ap()
    rbw_d = nc.dram_tensor("rbw", [N_RES, 3, DIM_MID, DIM_MID], bf16, kind="ExternalInput").ap()
    rbb_d = nc.dram_tensor("rbb", [N_RES, DIM_MID, 1], f32, kind="ExternalInput").ap()
    rbc1_d = nc.dram_tensor("rbc1", [N_RES, DIM_MID, DIM_MID], bf16, kind="ExternalInput").ap()
    rbc1b_d = nc.dram_tensor("rbc1b", [N_RES, DIM_MID, 1], f32, kind="ExternalInput").ap()
    c0w1_d = nc.dram_tensor("c0w1", [DIM_MID, DIM_MID], bf16, kind="ExternalInput").ap()
    c0b1_d = nc.dram_tensor("c0b1", [DIM_MID, 1], f32, kind="ExternalInput").ap()
    c0w2_d = nc.dram_tensor("c0w2", [DIM_MID, DIM_MID], bf16, kind="ExternalInput").ap()
    c0b2_d = nc.dram_tensor("c0b2", [DIM_MID, 1], f32, kind="ExternalInput").ap()
    finw_d = nc.dram_tensor("finw", [DIM_MID, DIM_OUT], bf16, kind="ExternalInput").ap()
    finb_d = nc.dram_tensor("finb", [DIM_OUT, 1], f32, kind="ExternalInput").ap()
    out_d = nc.dram_tensor("out", [4, DIM_OUT, NI], bf16, kind="ExternalOutput").ap()

    with tile.TileContext(nc) as tc:
        with (
            tc.tile_pool(name="const", bufs=1) as cpool,
            tc.tile_pool(name="stream", bufs=1) as spool,
            tc.tile_pool(name="x3p", bufs=2) as x3pool,
            tc.tile_pool(name="tcp", bufs=10) as tcpool,
            tc.tile_pool(name="tpp", bufs=6) as tppool,
            tc.tile_pool(name="syp", bufs=4) as sypool,
            tc.tile_pool(name="op", bufs=4) as opool,
            tc.tile_pool(name="psum", bufs=2, space="PSUM") as psum,
        ):
            # ---- constants ----
            w03q = cpool.tile([DIM_MID, DIM_MID], bf16)
            rbw = cpool.tile([DIM_MID, N_RES * 3, DIM_MID], bf16)
            rbc1 = cpool.tile([DIM_MID, N_RES, DIM_MID], bf16)
            c0w1 = cpool.tile([DIM_MID, DIM_MID], bf16)
            c0w2 = cpool.tile([DIM_MID, DIM_MID], bf16)
            finw = cpool.tile([DIM_MID, DIM_OUT], bf16)
            b0 = cpool.tile([DIM_MID, 1], f32)
            rbb = cpool.tile([DIM_MID, N_RES], f32)
            rbc1b = cpool.tile([DIM_MID, N_RES], f32)
            c0b1 = cpool.tile([DIM_MID, 1], f32)
            c0b2 = cpool.tile([DIM_MID, 1], f32)
            finb = cpool.tile([DIM_OUT, 1], f32)
            pre = cpool.tile([DIM_MID, 1], f32)

            rings = [spool.tile([DIM_MID, BUF], bf16, name=f"ring{d}")
                     for d in range(4)]

            # ---- startup: first-needed first ----
            for j in range(4):
                nc.sync.dma_start(w03q[32 * j:32 * j + 6, :], w03_d[6 * j:6 * j + 6, :])
            nc.sync.dma_start(b0[:], b0_d)
            # ACT silu table preload (overlaps the input DMAs)
            nc.scalar.activation(pre[:], b0[:], SILU)

            def zero_ring(s_t, eng):
                eng.memset(s_t[:, 0:BUF].bitcast(mybir.dt.uint32), 0)

            def load_weights_rb():
                nc.sync.dma_start(rbw[:], rbw_d.rearrange("i k ci co -> ci (i k) co"))
                nc.sync.dma_start(rbb[:], rbb_d.rearrange("i p one -> p (i one)"))
                nc.sync.dma_start(rbc1[:], rbc1_d.rearrange("i ci co -> ci i co"))
                nc.sync.dma_start(rbc1b[:], rbc1b_d.rearrange("i p one -> p (i one)"))

            def load_weights_c0():
                nc.sync.dma_start(c0w1[:], c0w1_d)
                nc.sync.dma_start(c0w2[:], c0w2_d)
                nc.sync.dma_start(finw[:], finw_d)
                nc.sync.dma_start(c0b1[:], c0b1_d)
                nc.sync.dma_start(c0b2[:], c0b2_d)
                nc.sync.dma_start(finb[:], finb_d)

            def emit_x3(d, spread=False):
                x3_t = x3pool.tile([DIM_MID, NGRP * CHUNK], bf16, tag="x3")
                qs = ((nc.sync, nc.gpsimd, nc.scalar, nc.gpsimd) if spread
                      else (nc.sync, nc.gpsimd, nc.sync, nc.gpsimd))
                for g in range(NGRP):
                    for j in range(4):
                        qs[g].dma_start(
                            x3_t[32 * j:32 * j + 6, g * CHUNK:(g + 1) * CHUNK],
                            x3_d[d, 6 * j:6 * j + 6, g * CHUNK:(g + 1) * CHUNK])
                return x3_t

            def emit_dconv0(s_t, x3_t, g):
                # dconv0 group: 4 matmuls, K=6, row-tiled via tile_position
                ps = psum.tile([DIM_MID, GRP, 512], f32, tag="ps")
                for j in range(GRP):
                    rp = 32 * j
                    nc.tensor.matmul(
                        ps[:, j, 0:CHUNK],
                        w03q[rp:rp + 6, :],
                        x3_t[rp:rp + 6, g * CHUNK:(g + 1) * CHUNK],
                        tile_position=(rp, 0))
                nc.scalar.activation(_int8(s_t, g), ps[:, :, 0:CHUNK],
                                     SILU, bias=b0[:])

            def emit_final(d, s_t, g):
                # final 1x1 conv (64 out ch) for group g (4 chunks)
                ps = psum.tile([DIM_MID, GRP, 512], f32, tag="ps")
                for j in range(GRP):
                    nc.tensor.matmul(ps[0:DIM_OUT, j, 0:CHUNK], finw[:],
                                     _conv_rhs(s_t, 0, GRP * g + j))
                o_g = opool.tile([DIM_OUT, GELEM], bf16, tag="og")
                # GpSimd cannot read PSUM; ACT is the bottleneck: DVE it is
                nc.vector.tensor_scalar_add(o_g[:], ps[0:DIM_OUT, :, 0:CHUNK],
                                            finb[:])
                oq = (nc.sync, nc.gpsimd)[(d + g) % 2]
                oq.dma_start(out_d[d][:, g * GELEM:(g + 1) * GELEM], o_g[:])

            def dve_silu(ps_banks, bias_ap, dst):
                """4-pass custom-DVE silu of (psum + bias) -> dst[p, GELEM].

                A DVE tensor_copy stages PSUM to SBUF first so the psum
                tile frees early (only ACT/DVE may read PSUM on hw)."""
                sy = sypool.tile([DIM_MID, GELEM], f32, tag="sy")
                nc.vector.tensor_copy(
                    sy.rearrange("p (b n) -> p b n", b=GRP), ps_banks)
                wt = sypool.tile([DIM_MID, GELEM], f32, tag="sy")
                w2 = sypool.tile([DIM_MID, GELEM], f32, tag="sy")
                v = nc.vector
                v._custom_dve(PA, out=wt[:], in0=sy[:],
                              s0=bias_ap, s1=INV_M, imm2=W_CLAMP)
                v._custom_dve(PB, out=w2[:], in0=wt[:],
                              s0=P_C2, s1=P_C3, imm2=1.0)
                v._custom_dve(PC, out=wt[:], in0=w2[:],
                              s0=R_ALPHA_P, s1=R_BETA, imm2=2.0)
                v._custom_dve(PD, out=dst, in0=sy[:], in1=wt[:],
                              s0=bias_ap, s1=1.0)

            def make_pair(dd, s_t, li):
                """Task closures for layer-pair li, per group g (4 chunks):
                conv->tc (ACT silu), pw->tp (ACT or DVE silu), add (DVE)."""
                rb = li < N_RES
                i = li if rb else 0
                offs = OFFS[dd]
                b1 = rbb[:, i:i + 1] if rb else c0b1[:]
                b2 = rbc1b[:, i:i + 1] if rb else c0b2[:]
                w2 = rbc1[:, i, :] if rb else c0w2[:]
                on_dve = (li, dd) in DVE_PW
                tcs = {}

                def emit_c(g):
                    ps = psum.tile([DIM_MID, GRP, 512], f32, tag="ps")
                    for j in range(GRP):
                        c = GRP * g + j
                        if rb:
                            for k in range(3):
                                nc.tensor.matmul(
                                    ps[:, j, 0:CHUNK], rbw[:, 3 * i + k, :],
                                    _conv_rhs(s_t, offs[k], c),
                                    start=(k == 0), stop=(k == 2))
                        else:
                            nc.tensor.matmul(ps[:, j, 0:CHUNK], c0w1[:],
                                             _conv_rhs(s_t, 0, c))
                    tc_t = tcpool.tile([DIM_MID, GELEM], bf16, tag="tc")
                    tcs[g] = tc_t
                    nc.scalar.activation(
                        tc_t.rearrange("p (b n) -> p b n", b=GRP),
                        ps[:, :, 0:CHUNK], SILU, bias=b1)

                def emit_p(g):
                    tc_t = tcs.pop(g)
                    ps = psum.tile([DIM_MID, GRP, 512], f32, tag="ps")
                    for j in range(GRP):
                        nc.tensor.matmul(
                            ps[:, j, 0:CHUNK], w2,
                            tc_t[:, j * CHUNK:(j + 1) * CHUNK])
                    tp_t = tppool.tile([DIM_MID, GELEM], bf16, tag="tp")
                    if on_dve:
                        dve_silu(ps[:, :, 0:CHUNK], b2, tp_t[:])
                    else:
                        nc.scalar.activation(
                            tp_t.rearrange("p (b n) -> p b n", b=GRP),
                            ps[:, :, 0:CHUNK], SILU, bias=b2)
                    nc.vector.tensor_add(
                        _int8(s_t, g), _int8(s_t, g),
                        tp_t.rearrange("p (i h w) -> p i h w", i=8, h=H, w=W))

                return emit_c, emit_p

            # conv (C) tasks lead pw (P) tasks by 2 psum-tile groups; all
            # four directions' slots interleave so the 2-tile psum
            # ping-pong always has other-direction work in flight.
            PAIR_SLOTS = ("C0", "C1", "C2", "C3", "P0", "P1", "P2", "P3")

            def emit_pair_zip(lis, post_p=None):
                cps = [make_pair(dd, rings[dd], li) for dd, li in lis]
                for slot in PAIR_SLOTS:
                    g = int(slot[1:])
                    for pi, (emit_c, emit_p) in enumerate(cps):
                        if slot[0] == "C":
                            emit_c(g)
                        else:
                            emit_p(g)
                            if post_p is not None:
                                post_p(lis[pi][0], g)

            # ---- startup ----
            x3s = [None] * 4
            x3s[0] = emit_x3(0, spread=True)
            zero_ring(rings[0], nc.vector)
            zero_ring(rings[1], nc.gpsimd)
            load_weights_rb()
            x3s[1] = emit_x3(1, spread=True)
            zero_ring(rings[2], nc.vector)
            zero_ring(rings[3], nc.gpsimd)
            load_weights_c0()
            for g in range(NGRP):
                emit_dconv0(rings[0], x3s[0], g)
            # dir 1's dconv0 woven into dir 0's first pair so the psum
            # ping-pong never idles during the solo stretch
            emit_pair_zip([(0, 0)],
                          post_p=lambda d, g: emit_dconv0(rings[1], x3s[1], g))

            # ---- main: staggered two-direction zip — dir 1 layer li runs
            # beside dir 0 layer li+1 so no pair ever runs solo; finals and
            # the next direction-pair's dconv0 fill the stagger tails.
            for li in range(N_RES):
                emit_pair_zip([(1, li), (0, li + 1)])
            x3s[2] = emit_x3(2)
            x3s[3] = emit_x3(3)

            def tail01(d, g):
                emit_final(0, rings[0], g)
                emit_dconv0(rings[2], x3s[2], g)
                emit_dconv0(rings[3], x3s[3], g)
            emit_pair_zip([(1, N_RES)], post_p=tail01)

            def tail1(d, g):
                if d == 3:
                    emit_final(1, rings[1], g)
            emit_pair_zip([(2, 0), (3, 0)], post_p=tail1)
            for li in range(1, N_RES + 1):
                if li == N_RES:
                    def tail_f(d, g):
                        if g > 0:
                            emit_final(d, rings[d], g - 1)
                    emit_pair_zip([(2, li), (3, li)], post_p=tail_f)
                else:
                    emit_pair_zip([(2, li), (3, li)])
            for d in (2, 3):
                emit_final(d, rings[d], NGRP - 1)

    nc.compile()
    return nc


def _bf16(a):
    import ml_dtypes
    return np.ascontiguousarray(np.asarray(a, np.float32)).astype(ml_dtypes.bfloat16)


def prep_shared_inputs(dconv0_w, dconv0_b, rb_dconv_w, rb_dconv_b, rb_c1_w,
                       rb_c1_b, c0_w1, c0_b1, c0_w2, c0_b2, final_w, final_b):
    f = np.float32
    w03 = np.ascontiguousarray(
        np.asarray(dconv0_w, f).transpose(0, 2, 1).reshape(6, DIM_MID))
    w03q = np.tile(w03, (4, 1))                       # [24, 128]
    finw = np.ascontiguousarray(np.asarray(final_w, f).T)   # [128, 64]
    return {
        "w03": _bf16(w03q),
        "b0": np.asarray(dconv0_b, f).reshape(DIM_MID, 1),
        "rbw": _bf16(np.asarray(rb_dconv_w, f).transpose(0, 1, 3, 2)),
        "rbb": np.asarray(rb_dconv_b, f).reshape(N_RES, DIM_MID, 1),
        "rbc1": _bf16(np.asarray(rb_c1_w, f).transpose(0, 2, 1)),
        "rbc1b": np.asarray(rb_c1_b, f).reshape(N_RES, DIM_MID, 1),
        "c0w1": _bf16(np.asarray(c0_w1, f).T),
        "c0b1": np.asarray(c0_b1, f).reshape(DIM_MID, 1),
        "c0w2": _bf16(np.asarray(c0_w2, f).T),
        "c0b2": np.asarray(c0_b2, f).reshape(DIM_MID, 1),
        "finw": _bf16(finw),
        "finb": np.asarray(final_b, f).reshape(DIM_OUT, 1),
    }


def prep_x3(x_shard):
    """[NB, 2, 15, 15] -> [4, 24, 1800]: pre-shifted interior copies, chunk
    c=4g+j of direction d at partition block j (rows 6j..6j+5), cols g*450."""
    P = np.zeros((NB, C_IN, H + 2, W + 2), np.float32)
    P[:, :, 1:16, 1:16] = x_shard
    x3 = np.empty((4, 6, NI), np.float32)
    for dd, taps in enumerate(POS):
        for k, (sr, sc) in enumerate(taps):
            sh = P[:, :, sr:sr + H, sc:sc + W]
            x3[dd, 2 * k:2 * k + 2] = sh.transpose(1, 0, 2, 3).reshape(C_IN, NI)
    x3q = np.empty((4, 24, NGRP * CHUNK), np.float32)
    for dd in range(4):
        o = x3[dd].reshape(6, 16, CHUNK)
        for j in range(4):
            x3q[dd, 6 * j:6 * j + 6, :] = o[:, j::4, :].reshape(6, NGRP * CHUNK)
    return _bf16(x3q)


_CACHE = {}


def kernel(**inputs):
    if "nc" not in _CACHE:
        _CACHE["nc"] = build_program()
    nc = _CACHE["nc"]

    x = np.asarray(inputs["x"], np.float32)
    shared = prep_shared_inputs(**{k: v for k, v in inputs.items() if k != "x"})

    in_maps = []
    for c in range(N_CORES):
        m = dict(shared)
        m["x3"] = prep_x3(x[c * NB:(c + 1) * NB])
        in_maps.append(m)

    def run_once():
        res = run_bass_kernel_spmd(nc, in_maps, core_ids=list(range(N_CORES)))
        out = np.empty((B, 4, DIM_OUT, H, W), np.float32)
        for c in range(N_CORES):
            oc = res.results[c]["out"].astype(np.float32).reshape(
                4, DIM_OUT, NB, H, W)
            out[c * NB:(c + 1) * NB] = oc.transpose(2, 0, 1, 3, 4)
        return out

    # rare hardware/transport flakes have produced a bad run; execute twice
    # and only accept agreeing results (third run breaks a mismatch)
    a = run_once()
    b = run_once()
    if np.allclose(a, b, rtol=1e-3, atol=1e-3):
        return a
    c = run_once()
    return c if np.allclose(b, c, rtol=1e-3, atol=1e-3) else a
